# revision 33
# baseline (speedup 1.0000x reference)
"""GraphSAGE (3-layer, mean aggregation) on 8 Trainium2 NeuronCores.

Strategy (1D graph partitioning, nodes sharded by row across 8 cores):
  - Core c owns nodes [c*12500, (c+1)*12500); edges partitioned by dst.
  - Per layer l:  Y = h_local @ Wl  (dense, fp32 PE)  -> stored bf16
                  AllGather Y -> Y_full (bf16, replicated per core)
                  gather Y_full[src] rows for this core's edges with
                  dma_gather (int16 indices, 4 source ranges of 25088 rows),
                  then segment-sum by dst via one-hot selection matmuls
                  accumulated in PSUM (fp32)
                  h_new = relu(seg_sum * inv_deg + h_local @ Wr + b)
  - Segment-sum-by-matmul: for a block of 128 edges, S[e, j] = (dst_off[e]==j)
    built on DVE via is_equal against an iota row; PSUM accumulates
    S^T @ G over the window's blocks.

Edges are grouped host-side by (dst-window, src-range); each (window, range)
run is padded to whole 128-edge blocks with slot index 0 (gathers a garbage
row that the all-zero one-hot column ignores). Block counts are maxed across
cores so all 8 cores run one SPMD program.
"""

import os

import numpy as np
import ml_dtypes

P = 128
NCORES = 8
N_NODES = 100000
NLOC = N_NODES // NCORES            # 12500 nodes per core
NW = (NLOC + P - 1) // P            # 98 dst windows per core
NLOCP = NW * P                      # 12544 (padded local nodes)
NFULLP = NCORES * NLOCP             # 100352 (padded global rows)
NRANGE = 4
RSIZE = NFULLP // NRANGE            # 25088 rows per gather range (int16-safe)
NSW = 8                             # windows per super-window (gather batch)
DIMS = [(128, 128), (128, 128), (128, 64)]
GCH = 128                           # gathered feature columns (Y3 zero-padded)
OUT_CH = 64

LAST_EXEC_TIME_NS = None
LAST_RESULTS = None


def _sw_groups(nw, nsw):
    return [list(range(a, min(a + nsw, nw))) for a in range(0, nw, nsw)]


def _build_program(layout, nw=NW, nlocp=NLOCP, nfullp=NFULLP, ncores=NCORES,
                   dims=DIMS, debug=False, model_mode=False, ablate=()):
    """layout: dict with nblk [nw][4], plus derived column/idx offsets.

    ablate: subset of {"gather", "agg", "dense", "collective"} — skip that
    phase (produces wrong results; for timing attribution only).
    """
    import concourse.bacc as bacc
    import concourse.bass as bass
    import concourse.mybir as mybir
    import concourse.tile as tile
    from concourse.masks import make_identity

    dt = mybir.dt
    AF = mybir.ActivationFunctionType
    OP = mybir.AluOpType
    out_ch = dims[-1][1]
    rsize = nfullp // NRANGE

    runs = layout["runs"]              # per sw: list of (r, col_start, nblks)
    spans = layout["spans"]            # [nw][NRANGE] (b0, b1) global cols
    sw_groups = layout["sw_groups"]
    total_cols = layout["total_cols"]
    max_sw_cols = layout["max_sw_cols"]
    max_span = layout["max_span"]
    sw_col_start = layout["sw_col_start"]
    nsw = len(sw_groups[0])

    nc = bacc.Bacc("TRN2", target_bir_lowering=False, debug=False,
                   num_devices=ncores, num_swdge_queues=2)

    x_in = nc.dram_tensor("x_local", [nlocp, dims[0][0]], dt.bfloat16,
                          kind="ExternalInput")
    wcat_in = [nc.dram_tensor(f"wcat{l}", [dims[l][0], 2 * dims[l][1]],
                              dt.bfloat16, kind="ExternalInput")
               for l in range(3)]
    bbc_in = [nc.dram_tensor(f"bbc{l}", [P, dims[l][1]], dt.float32,
                             kind="ExternalInput") for l in range(3)]
    idx16_in = nc.dram_tensor("idx16", [P, total_cols * 8], dt.int16,
                              kind="ExternalInput")
    dstw_in = nc.dram_tensor("dstw", [P, total_cols, nsw], dt.bfloat16,
                             kind="ExternalInput")
    invd_in = nc.dram_tensor("invd", [P, nw], dt.float32,
                             kind="ExternalInput")
    iota_in = nc.dram_tensor("iota", [P, 1, P], dt.bfloat16,
                             kind="ExternalInput")
    h_out = nc.dram_tensor("h_out", [nlocp, out_ch], dt.float32,
                           kind="ExternalOutput")
    dbg = {}
    if debug:
        for l in range(3):
            dbg[f"y_full_d{l}"] = nc.dram_tensor(
                f"y_full_d{l}", [nfullp, GCH], dt.bfloat16,
                kind="ExternalOutput")

    with tile.TileContext(nc) as tc:
        with (
            tc.tile_pool(name="const", bufs=1) as cpool,
            tc.tile_pool(name="dram", bufs=1, space="DRAM") as dpool,
            tc.tile_pool(name="hload", bufs=3) as hpool,
            tc.tile_pool(name="htr", bufs=3) as htpool,
            tc.tile_pool(name="yt", bufs=3) as ypool,
            tc.tile_pool(name="gat", bufs=2) as gpool,
            tc.tile_pool(name="idx", bufs=2) as ipool,
            tc.tile_pool(name="dwp", bufs=2) as dpool2,
            tc.tile_pool(name="sel", bufs=4) as spool,
            tc.tile_pool(name="epi", bufs=4) as epool,
            tc.tile_pool(name="pst", bufs=2, space="PSUM") as pt_pool,
            tc.tile_pool(name="psm", bufs=2, space="PSUM") as pmm_pool,
            tc.tile_pool(name="psa", bufs=4, space="PSUM") as pa_pool,
        ):
            ident = cpool.tile([P, P], dt.bfloat16)
            make_identity(nc, ident[:])
            iota_sb = cpool.tile([P, 1, P], dt.bfloat16)
            nc.sync.dma_start(iota_sb[:], iota_in[:, :, :])
            invd_sb = cpool.tile([P, nw], dt.float32)
            nc.sync.dma_start(invd_sb[:], invd_in[:, :])
            wc_sb = []
            bb_sb = []
            for l in range(3):
                w_t = cpool.tile([dims[l][0], 2 * dims[l][1]], dt.bfloat16,
                                 name=f"wc{l}")
                nc.sync.dma_start(w_t[:], wcat_in[l][:, :])
                wc_sb.append(w_t)
                b_t = cpool.tile([P, dims[l][1]], dt.float32, name=f"bb{l}")
                nc.sync.dma_start(b_t[:], bbc_in[l][:, :])
                bb_sb.append(b_t)
            r_res = cpool.tile([P, nw, dims[0][1]], dt.bfloat16)
            if "dense" in ablate:
                nc.vector.memset(r_res[:], 0.0)
            # h for layers 1/2 lives in SBUF (bf16), window-sliced: the
            # next layer's dense phase reads it without any DRAM round
            # trip and can overlap the previous layer's aggregation.
            h_sb = [cpool.tile([P, nw, dims[0][1]], dt.bfloat16,
                               name=f"hsb{i}") for i in range(2)]
            if "agg" in ablate:
                for t in h_sb:
                    nc.vector.memset(t[:], 0.0)

            y_locs = [dpool.tile([nlocp, GCH], dt.bfloat16,
                                 name=f"y_loc{l}") for l in range(3)]
            y_fulls = [dpool.tile([nfullp, GCH], dt.bfloat16,
                                  addr_space="Shared", name=f"y_full{l}")
                       for l in range(3)]

            def dense_w(l, i, h_ap):
                # Y_w = h_w @ Wl (-> bf16 y_loc), r_res_w = h_w @ Wr + b
                din, dout = dims[l]
                t_ps = pt_pool.tile([P, P], dt.bfloat16, tag="t_ps")
                nc.tensor.transpose(t_ps[:din, :], h_ap, ident[:])
                hT = htpool.tile([P, P], dt.bfloat16, tag="hT")
                nc.vector.tensor_copy(hT[:din, :], t_ps[:din, :])
                mm = pmm_pool.tile([P, 2 * dout], dt.float32, tag="mm")
                nc.tensor.matmul(mm[:], lhsT=hT[:din, :],
                                 rhs=wc_sb[l][:, :], start=True, stop=True)
                y_t = ypool.tile([P, GCH], dt.bfloat16, tag="y_t")
                nc.scalar.activation(y_t[:, :dout], mm[:, :dout], AF.Copy)
                if dout < GCH:
                    nc.vector.memset(y_t[:, dout:], 0.0)
                nc.sync.dma_start(y_locs[l][i * P:(i + 1) * P, :], y_t[:])
                nc.vector.tensor_tensor(out=r_res[:, i, :dout],
                                        in0=mm[:, dout:2 * dout],
                                        in1=bb_sb[l][:, :], op=OP.add)

            for l in range(3):
                din, dout = dims[l]
                y_loc = y_locs[l]
                y_full = y_fulls[l]

                # ---- dense phase: layer 0 from x; layers 1/2 are emitted
                # inside the previous layer's epilogue (window-fused) so
                # they overlap that layer's gathers/aggregation ----
                if "dense" not in ablate and l == 0:
                    for i in range(nw):
                        h_t = hpool.tile([P, din], dt.bfloat16, tag="h_t")
                        nc.sync.dma_start(h_t[:],
                                          x_in[i * P:(i + 1) * P, :])
                        dense_w(0, i, h_t[:])

                # ---- AllGather Y (bf16) ----
                if model_mode or "collective" in ablate:
                    nc.sync.dma_start(y_full[0:nlocp, :], y_loc[:, :])
                else:
                    nc.gpsimd.collective_compute(
                        "AllGather", mybir.AluOpType.bypass,
                        replica_groups=[list(range(ncores))],
                        ins=[y_loc.opt()], outs=[y_full.opt()])
                if debug:
                    nc.sync.dma_start(dbg[f"y_full_d{l}"][:, :],
                                      y_full[:, :])

                # ---- aggregation phase ----
                for si, grp in enumerate(sw_groups):
                    c0 = sw_col_start[si]
                    sw_cols = sw_col_start[si + 1] - c0
                    g_t = gpool.tile([P, max_sw_cols, GCH], dt.bfloat16,
                                     tag="g_t")
                    if "gather" in ablate:
                        nc.vector.memset(g_t[:], 0.0)
                    i_t = ipool.tile([P, max_sw_cols * 8], dt.int16,
                                     tag="i_t")
                    nc.sync.dma_start(i_t[:, :sw_cols * 8],
                                      idx16_in[:, c0 * 8:(c0 + sw_cols) * 8])
                    d_t = dpool2.tile([P, max_sw_cols, nsw], dt.bfloat16,
                                      tag="d_t")
                    nc.sync.dma_start(d_t[:, :sw_cols, :],
                                      dstw_in[:, c0:c0 + sw_cols, :])
                    for (r, rc0, rblk) in runs[si]:
                        if rblk == 0 or "gather" in ablate:
                            continue
                        lc = rc0 - c0
                        # alternate the two SWDGE queues: descriptor
                        # generation for consecutive gathers runs on
                        # different Q7 contexts (~2.4x measured)
                        nc.gpsimd.dma_gather(
                            out_ap=g_t[:, lc:lc + rblk, :],
                            in_ap=y_full[r * rsize:(r + 1) * rsize, :],
                            idxs_ap=i_t[:, lc * 8:(lc + rblk) * 8],
                            num_idxs=rblk * P, num_idxs_reg=rblk * P,
                            elem_size=GCH, single_packet=False,
                            queue_num=(si * NRANGE + r) % 2)
                    if "agg" in ablate:
                        continue
                    for w in grp:
                        wi = w - grp[0]
                        agg = pa_pool.tile([P, GCH], dt.float32, tag="agg")
                        nb_w = sum(b1 - b0 for (b0, b1) in spans[w])
                        done = 0
                        for r in range(NRANGE):
                            b0, b1 = spans[w][r]
                            nb = b1 - b0
                            if nb == 0:
                                continue
                            lc = b0 - c0
                            s_t = spool.tile([P, max_span, P],
                                             dt.bfloat16, tag="s_t")
                            nc.vector.tensor_tensor(
                                out=s_t[:, :nb, :],
                                in0=iota_sb[:, :, :].to_broadcast(
                                    [P, nb, P]),
                                in1=d_t[:, lc:lc + nb, wi:wi + 1]
                                .to_broadcast([P, nb, P]),
                                op=OP.is_equal)
                            for k in range(nb):
                                nc.tensor.matmul(
                                    agg[:], lhsT=s_t[:, k, :],
                                    rhs=g_t[:, lc + k, :],
                                    start=(done == 0),
                                    stop=(done == nb_w - 1))
                                done += 1
                        if l < 2:
                            t_t = epool.tile([P, dout], dt.bfloat16,
                                             tag="t_t")
                            nc.scalar.activation(t_t[:], agg[:, :dout],
                                                 AF.Copy,
                                                 scale=invd_sb[:, w:w + 1])
                            o_t = epool.tile([P, dout], dt.bfloat16,
                                             tag="o_t")
                            nc.vector.tensor_tensor(out=o_t[:], in0=t_t[:],
                                                    in1=r_res[:, w, :dout],
                                                    op=OP.add)
                            nc.vector.tensor_scalar_max(
                                h_sb[l % 2][:, w, :dout], o_t[:], 0.0)
                            if "dense" not in ablate:
                                dense_w(l + 1, w,
                                        h_sb[l % 2][:, w, :dout])
                        else:
                            t_t = epool.tile([P, dout], dt.float32,
                                             tag="t_t")
                            nc.scalar.activation(t_t[:], agg[:, :dout],
                                                 AF.Copy,
                                                 scale=invd_sb[:, w:w + 1])
                            o_t = epool.tile([P, dout], dt.float32,
                                             tag="o_t")
                            nc.vector.tensor_tensor(out=o_t[:], in0=t_t[:],
                                                    in1=r_res[:, w, :dout],
                                                    op=OP.add)
                            nc.sync.dma_start(h_out[w * P:(w + 1) * P, :],
                                              o_t[:])


    nc.compile()
    return nc


def _preprocess(x, src, dst, ncores=NCORES, nloc=NLOC, nw=NW, nlocp=NLOCP,
                nsw=NSW):
    """Pack per-core edge/index arrays grouped by (super-window, src range).

    Edges are packed edge-granularly inside each (sw, range) run using
    SPMD-uniform per-(window, range) segment sizes (max over cores), so
    only the run total is padded to whole 128-slot blocks. A block may
    hold edges of several windows; the per-window one-hot selects them
    via per-window dstw planes (-1 for foreign slots).

    Returns (per_core input dicts, layout dict for _build_program).
    """
    bf16 = ml_dtypes.bfloat16
    nfullp = ncores * nlocp
    rsize = nfullp // NRANGE

    order = np.argsort(dst, kind="stable")
    src_s = src[order].astype(np.int64)
    dst_s = dst[order].astype(np.int64)
    bounds = np.searchsorted(dst_s, np.arange(ncores + 1) * nloc)

    cores = []
    cnts = np.zeros((ncores, nw, NRANGE), np.int64)
    for c in range(ncores):
        lo, hi = bounds[c], bounds[c + 1]
        s = src_s[lo:hi]
        lcl = dst_s[lo:hi] - c * nloc
        w = lcl // P
        sowner = s // nloc
        s_pad = sowner * nlocp + (s - sowner * nloc)
        rix = s_pad // rsize
        key = w * NRANGE + rix
        o2 = np.lexsort((s_pad, key))
        s_pad, lcl, w, rix, key = (s_pad[o2], lcl[o2], w[o2], rix[o2],
                                   key[o2])
        cnts[c] = np.bincount(key, minlength=nw * NRANGE)\
            .reshape(nw, NRANGE)
        cores.append((s_pad, lcl, w, rix, key))

    # SPMD-uniform segment sizes: max edge count over cores per (w, r)
    seg = cnts.max(axis=0).astype(np.int64)        # [nw, NRANGE]
    seg[:, 0] = np.maximum(seg[:, 0], 1)           # keep every window alive

    sw_groups = _sw_groups(nw, nsw)
    seg_start = np.zeros((nw, NRANGE), np.int64)   # slot offset within run
    run_col = np.zeros((len(sw_groups), NRANGE), np.int64)
    runs = []
    spans = [[None] * NRANGE for _ in range(nw)]   # (b0, b1) global cols
    sw_col_start = [0]
    gc = 0
    for si, grp in enumerate(sw_groups):
        sw_runs = []
        for r in range(NRANGE):
            tot = 0
            for w in grp:
                seg_start[w, r] = tot
                tot += seg[w, r]
            nb = (tot + P - 1) // P
            run_col[si, r] = gc
            for w in grp:
                s0, s1 = seg_start[w, r], seg_start[w, r] + seg[w, r]
                if s1 > s0:
                    spans[w][r] = (int(gc + s0 // P),
                                   int(gc + (s1 + P - 1) // P))
                else:
                    spans[w][r] = (int(gc), int(gc))
            sw_runs.append((r, int(gc), int(nb)))
            gc += nb
        runs.append(sw_runs)
        sw_col_start.append(int(gc))
    total_cols = int(gc)
    max_sw_cols = max(sw_col_start[i + 1] - sw_col_start[i]
                      for i in range(len(sw_groups)))
    max_span = max(b1 - b0 for row in spans for (b0, b1) in row)
    layout = {
        "seg": seg.tolist(),
        "runs": runs,
        "spans": spans,
        "sw_groups": sw_groups,
        "sw_col_start": sw_col_start,
        "total_cols": total_cols,
        "max_sw_cols": int(max_sw_cols),
        "max_span": int(max_span),
    }

    per_core = []
    for c in range(ncores):
        s_pad, lcl, w, rix, key = cores[c]
        cnt = cnts[c]
        starts = np.zeros(nw * NRANGE, np.int64)
        starts[1:] = np.cumsum(cnt.ravel())[:-1]
        j = np.arange(len(lcl)) - starts[key]
        sw = w // nsw
        slot_in_run = seg_start[w, rix] + j
        rcol = run_col[sw, rix]
        col = rcol + slot_in_run // P
        pp = slot_in_run % P
        # per-window dstw planes: plane wi holds the dst offset within
        # window (sw*nsw + wi) for that window's slots, -1 elsewhere
        dstw = np.full((P, total_cols, nsw), -1.0, np.float32)
        dstw[pp, col, w % nsw] = (lcl % P).astype(np.float32)
        i16col = rcol * 8 + slot_in_run // 16
        i16row = slot_in_run % 16
        idx16 = np.zeros((16, total_cols * 8), np.int16)
        idx16[i16row, i16col] = (s_pad - rix * rsize).astype(np.int16)
        idx16 = np.tile(idx16, (8, 1))
        deg = np.bincount(lcl, minlength=nlocp).astype(np.float32)
        invd = (1.0 / np.maximum(deg, 1.0)).reshape(nw, P).T.copy()
        x_pad = np.zeros((nlocp, x.shape[1]), np.float32)
        x_pad[:nloc] = x[c * nloc:(c + 1) * nloc]
        per_core.append({
            "x_local": x_pad,
            "idx16": idx16,
            "dstw": dstw.astype(bf16),
            "invd": invd.astype(np.float32),
        })
    return per_core, layout


def _run_pjrt(nc, in_maps, n_cores, bench_iters=0):
    """Execute the Bass program on the NeuronCores via PJRT/axon.

    Mirrors concourse.bass2jax.run_bass_via_pjrt, with an optional timing
    loop: inputs are pre-placed on device so repeated calls measure
    execute time (plus dispatch overhead) rather than host transfers.
    Returns (per_core_results, best_ns or None).
    """
    import time
    import jax
    import concourse.mybir as mybir
    from concourse.bass2jax import (_bass_exec_p, install_neuronx_cc_hook,
                                    partition_id_tensor)
    from jax.experimental.shard_map import shard_map
    from jax.sharding import Mesh, NamedSharding, PartitionSpec

    install_neuronx_cc_hook()

    partition_name = (nc.partition_id_tensor.name
                      if nc.partition_id_tensor else None)
    in_names, out_names, out_avals, zero_outs = [], [], [], []
    for alloc in nc.m.functions[0].allocations:
        if not isinstance(alloc, mybir.MemoryLocationSet):
            continue
        name = alloc.memorylocations[0].name
        if alloc.kind == "ExternalInput":
            if name != partition_name:
                in_names.append(name)
        elif alloc.kind == "ExternalOutput":
            shape = tuple(alloc.tensor_shape)
            dtype = mybir.dt.np(alloc.dtype)
            out_names.append(name)
            out_avals.append(jax.core.ShapedArray(shape, dtype))
            zero_outs.append(np.zeros(shape, dtype))
    n_params = len(in_names)
    n_outs = len(out_avals)
    in_names.extend(out_names)
    if partition_name is not None:
        in_names.append(partition_name)

    def _body(*args):
        operands = list(args)
        if partition_name is not None:
            operands.append(partition_id_tensor())
        return tuple(_bass_exec_p.bind(
            *operands,
            out_avals=tuple(out_avals),
            in_names=tuple(in_names),
            out_names=tuple(out_names),
            lowering_input_output_aliases=(),
            sim_require_finite=True,
            sim_require_nnan=True,
            nc=nc,
        ))

    devices = jax.devices()[:n_cores]
    assert len(devices) >= n_cores, devices
    mesh = Mesh(np.asarray(devices), ("core",))
    in_specs = (PartitionSpec("core"),) * (n_params + n_outs)
    out_specs = (PartitionSpec("core"),) * n_outs
    sharded = jax.jit(
        shard_map(_body, mesh=mesh, in_specs=in_specs, out_specs=out_specs,
                  check_rep=False),
        keep_unused=True)

    per_core = [[np.asarray(m[name]) for name in in_names[:n_params]]
                for m in in_maps]
    concat_in = [np.concatenate([per_core[c][i] for c in range(n_cores)],
                                axis=0) for i in range(n_params)]
    concat_zeros = [np.zeros((n_cores * z.shape[0], *z.shape[1:]), z.dtype)
                    for z in zero_outs]

    sharding = NamedSharding(mesh, PartitionSpec("core"))
    dev_in = [jax.device_put(a, sharding) for a in concat_in]
    dev_zeros = [jax.device_put(z, sharding) for z in concat_zeros]

    out_arrs = sharded(*dev_in, *dev_zeros)
    out_arrs = [np.asarray(o) for o in out_arrs]

    best_ns = None
    if bench_iters:
        # Device-time measurement through the axon tunnel: issue a chain
        # of async dispatches where each call's (unused) zero-output
        # operands come from the previous call's outputs. The device
        # serializes the NEFF executions while dispatch RPCs pipeline
        # (marginal dispatch cost ~0.2 ms/call at depth >=32), so
        # (t_deep - t_shallow) / (n_deep - n_shallow) isolates per-exec
        # device time and cancels the ~72 ms round-trip.
        def chain(n):
            outs = tuple(dev_zeros)
            t0 = time.perf_counter()
            for _ in range(n):
                outs = sharded(*dev_in, *outs)
            for r in outs:
                r.block_until_ready()
            return time.perf_counter() - t0

        n1, n2 = 12, 12 + max(24, 2 * bench_iters)
        chain(4)  # warm
        for _ in range(3):
            t1 = chain(n1)
            t2 = chain(n2)
            dt_ns = (t2 - t1) / (n2 - n1) * 1e9
            best_ns = dt_ns if best_ns is None else min(best_ns, dt_ns)

    results = [
        {name: out_arrs[i].reshape(n_cores, *out_avals[i].shape)[c]
         for i, name in enumerate(out_names)}
        for c in range(n_cores)
    ]
    return results, best_ns


def kernel(x, edge_index, Wl0, Wr0, b0, Wl1, Wr1, b1, Wl2, Wr2, b2):
    global LAST_EXEC_TIME_NS, LAST_RESULTS

    bf16 = ml_dtypes.bfloat16
    x = np.ascontiguousarray(np.asarray(x, np.float32))
    ei = np.asarray(edge_index)
    src = ei[0].astype(np.int64)
    dst = ei[1].astype(np.int64)

    per_core, layout = _preprocess(x, src, dst)

    Ws = [(np.asarray(Wl0, np.float32), np.asarray(Wr0, np.float32),
           np.asarray(b0, np.float32)),
          (np.asarray(Wl1, np.float32), np.asarray(Wr1, np.float32),
           np.asarray(b1, np.float32)),
          (np.asarray(Wl2, np.float32), np.asarray(Wr2, np.float32),
           np.asarray(b2, np.float32))]
    shared = {}
    for l, (Wl, Wr, b) in enumerate(Ws):
        shared[f"wcat{l}"] = np.ascontiguousarray(
            np.concatenate([Wl, Wr], axis=1).astype(bf16))
        shared[f"bbc{l}"] = np.ascontiguousarray(
            np.tile(b[None, :], (P, 1)).astype(np.float32))
    shared["iota"] = np.tile(np.arange(P, dtype=np.float32)[None, None, :],
                             (P, 1, 1)).astype(bf16)

    in_maps = [{**pc, **shared} for pc in per_core]
    for m in in_maps:
        m["x_local"] = m["x_local"].astype(bf16)

    nc = _build_program(layout)
    bench_iters = int(os.environ.get("GSAGE_BENCH_ITERS", "8"))
    results, best_ns = _run_pjrt(nc, in_maps, NCORES,
                                 bench_iters=bench_iters)
    LAST_EXEC_TIME_NS = best_ns
    LAST_RESULTS = results

    out = np.empty((N_NODES, OUT_CH), np.float32)
    for c in range(NCORES):
        out[c * NLOC:(c + 1) * NLOC] = results[c]["h_out"][:NLOC]
    return out



# revision 37
# speedup vs baseline: 1.0615x; 1.0615x over previous
"""GraphSAGE (3-layer, mean aggregation) on 8 Trainium2 NeuronCores.

Strategy (1D graph partitioning, nodes sharded by row across 8 cores):
  - Core c owns nodes [c*12500, (c+1)*12500); edges partitioned by dst.
  - Per layer l:  Y = h_local @ Wl  (dense, fp32 PE)  -> stored bf16
                  AllGather Y -> Y_full (bf16, replicated per core)
                  gather Y_full[src] rows for this core's edges with
                  dma_gather (int16 indices, 4 source ranges of 25088 rows),
                  then segment-sum by dst via one-hot selection matmuls
                  accumulated in PSUM (fp32)
                  h_new = relu(seg_sum * inv_deg + h_local @ Wr + b)
  - Segment-sum-by-matmul: for a block of 128 edges, S[e, j] = (dst_off[e]==j)
    built on DVE via is_equal against an iota row; PSUM accumulates
    S^T @ G over the window's blocks.

Edges are grouped host-side by (dst-window, src-range); each (window, range)
run is padded to whole 128-edge blocks with slot index 0 (gathers a garbage
row that the all-zero one-hot column ignores). Block counts are maxed across
cores so all 8 cores run one SPMD program.
"""

import os

import numpy as np
import ml_dtypes

P = 128
NCORES = 8
N_NODES = 100000
NLOC = N_NODES // NCORES            # 12500 nodes per core
NW = (NLOC + P - 1) // P            # 98 dst windows per core
NLOCP = NW * P                      # 12544 (padded local nodes)
NFULLP = NCORES * NLOCP             # 100352 (padded global rows)
NRANGE = 4
RSIZE = NFULLP // NRANGE            # 25088 rows per gather range (int16-safe)
NSW = 8                             # windows per super-window (gather batch)
DIMS = [(128, 128), (128, 128), (128, 64)]
GCH = 128                           # gathered feature columns (Y3 zero-padded)
OUT_CH = 64

LAST_EXEC_TIME_NS = None
LAST_RESULTS = None


def _sw_groups(nw, nsw):
    return [list(range(a, min(a + nsw, nw))) for a in range(0, nw, nsw)]


def _build_program(layout, nw=NW, nlocp=NLOCP, nfullp=NFULLP, ncores=NCORES,
                   dims=DIMS, debug=False, model_mode=False, ablate=(),
                   fuse_dense=True):
    """layout: dict with nblk [nw][4], plus derived column/idx offsets.

    ablate: subset of {"gather", "agg", "dense", "collective"} — skip that
    phase (produces wrong results; for timing attribution only).
    """
    import concourse.bacc as bacc
    import concourse.bass as bass
    import concourse.mybir as mybir
    import concourse.tile as tile
    from concourse.masks import make_identity

    dt = mybir.dt
    AF = mybir.ActivationFunctionType
    OP = mybir.AluOpType
    out_ch = dims[-1][1]
    rsize = nfullp // NRANGE

    runs = layout["runs"]              # per sw: list of (r, col_start, nblks)
    spans = layout["spans"]            # [nw][NRANGE] (b0, b1) global cols
    sw_groups = layout["sw_groups"]
    total_cols = layout["total_cols"]
    max_sw_cols = layout["max_sw_cols"]
    max_span = layout["max_span"]
    sw_col_start = layout["sw_col_start"]
    nsw = len(sw_groups[0])

    nc = bacc.Bacc("TRN2", target_bir_lowering=False, debug=False,
                   num_devices=ncores, num_swdge_queues=2)

    x_in = nc.dram_tensor("x_local", [nlocp, dims[0][0]], dt.bfloat16,
                          kind="ExternalInput")
    wcat_in = [nc.dram_tensor(f"wcat{l}", [dims[l][0], 2 * dims[l][1]],
                              dt.bfloat16, kind="ExternalInput")
               for l in range(3)]
    bbc_in = [nc.dram_tensor(f"bbc{l}", [P, dims[l][1]], dt.float32,
                             kind="ExternalInput") for l in range(3)]
    idx16_in = nc.dram_tensor("idx16", [P, total_cols * 8], dt.int16,
                              kind="ExternalInput")
    dstw_in = nc.dram_tensor("dstw", [P, total_cols, nsw], dt.bfloat16,
                             kind="ExternalInput")
    invd_in = nc.dram_tensor("invd", [P, nw], dt.float32,
                             kind="ExternalInput")
    iota_in = nc.dram_tensor("iota", [P, 1, P], dt.bfloat16,
                             kind="ExternalInput")
    h_out = nc.dram_tensor("h_out", [nlocp, out_ch], dt.float32,
                           kind="ExternalOutput")
    dbg = {}
    if debug:
        for l in range(3):
            dbg[f"y_full_d{l}"] = nc.dram_tensor(
                f"y_full_d{l}", [nfullp, GCH], dt.bfloat16,
                kind="ExternalOutput")

    with tile.TileContext(nc) as tc:
        with (
            tc.tile_pool(name="const", bufs=1) as cpool,
            tc.tile_pool(name="dram", bufs=1, space="DRAM") as dpool,
            tc.tile_pool(name="hload", bufs=3) as hpool,
            tc.tile_pool(name="htr", bufs=3) as htpool,
            tc.tile_pool(name="yt", bufs=3) as ypool,
            tc.tile_pool(name="gat", bufs=2) as gpool,
            tc.tile_pool(name="idx", bufs=2) as ipool,
            tc.tile_pool(name="dwp", bufs=2) as dpool2,
            tc.tile_pool(name="sel", bufs=4) as spool,
            tc.tile_pool(name="epi", bufs=4) as epool,
            tc.tile_pool(name="pst", bufs=2, space="PSUM") as pt_pool,
            tc.tile_pool(name="psm", bufs=2, space="PSUM") as pmm_pool,
            tc.tile_pool(name="psa", bufs=4, space="PSUM") as pa_pool,
        ):
            ident = cpool.tile([P, P], dt.bfloat16)
            make_identity(nc, ident[:])
            iota_sb = cpool.tile([P, 1, P], dt.bfloat16)
            nc.sync.dma_start(iota_sb[:], iota_in[:, :, :])
            invd_sb = cpool.tile([P, nw], dt.float32)
            nc.sync.dma_start(invd_sb[:], invd_in[:, :])
            wc_sb = []
            bb_sb = []
            for l in range(3):
                w_t = cpool.tile([dims[l][0], 2 * dims[l][1]], dt.bfloat16,
                                 name=f"wc{l}")
                nc.sync.dma_start(w_t[:], wcat_in[l][:, :])
                wc_sb.append(w_t)
                b_t = cpool.tile([P, dims[l][1]], dt.float32, name=f"bb{l}")
                nc.sync.dma_start(b_t[:], bbc_in[l][:, :])
                bb_sb.append(b_t)
            r_res = cpool.tile([P, nw, dims[0][1]], dt.bfloat16)
            if "dense" in ablate:
                nc.vector.memset(r_res[:], 0.0)
            # h for layers 1/2 lives in SBUF (bf16), window-sliced: the
            # next layer's dense phase reads it without any DRAM round
            # trip and can overlap the previous layer's aggregation.
            h_sb = [cpool.tile([P, nw, dims[0][1]], dt.bfloat16,
                               name=f"hsb{i}") for i in range(2)]
            if "agg" in ablate:
                for t in h_sb:
                    nc.vector.memset(t[:], 0.0)

            y_locs = [dpool.tile([nlocp, GCH], dt.bfloat16,
                                 name=f"y_loc{l}") for l in range(3)]
            y_fulls = [dpool.tile([nfullp, GCH], dt.bfloat16,
                                  addr_space="Shared", name=f"y_full{l}")
                       for l in range(3)]

            def dense_w(l, i, h_ap):
                # Y_w = h_w @ Wl (-> bf16 y_loc), r_res_w = h_w @ Wr + b
                din, dout = dims[l]
                t_ps = pt_pool.tile([P, P], dt.bfloat16, tag="t_ps")
                nc.tensor.transpose(t_ps[:din, :], h_ap, ident[:])
                hT = htpool.tile([P, P], dt.bfloat16, tag="hT")
                nc.vector.tensor_copy(hT[:din, :], t_ps[:din, :])
                mm = pmm_pool.tile([P, 2 * dout], dt.float32, tag="mm")
                nc.tensor.matmul(mm[:], lhsT=hT[:din, :],
                                 rhs=wc_sb[l][:, :], start=True, stop=True)
                y_t = ypool.tile([P, GCH], dt.bfloat16, tag="y_t")
                nc.scalar.activation(y_t[:, :dout], mm[:, :dout], AF.Copy)
                if dout < GCH:
                    nc.vector.memset(y_t[:, dout:], 0.0)
                nc.sync.dma_start(y_locs[l][i * P:(i + 1) * P, :], y_t[:])
                nc.vector.tensor_tensor(out=r_res[:, i, :dout],
                                        in0=mm[:, dout:2 * dout],
                                        in1=bb_sb[l][:, :], op=OP.add)

            for l in range(3):
                din, dout = dims[l]
                y_loc = y_locs[l]
                y_full = y_fulls[l]

                # ---- dense phase: layer 0 from x; layers 1/2 are emitted
                # inside the previous layer's epilogue (window-fused) so
                # they overlap that layer's gathers/aggregation ----
                if "dense" not in ablate and l == 0:
                    for i in range(nw):
                        h_t = hpool.tile([P, din], dt.bfloat16, tag="h_t")
                        nc.sync.dma_start(h_t[:],
                                          x_in[i * P:(i + 1) * P, :])
                        dense_w(0, i, h_t[:])
                if "dense" not in ablate and l > 0 and not fuse_dense:
                    for i in range(nw):
                        dense_w(l, i, h_sb[(l - 1) % 2][:, i, :din])

                # ---- AllGather Y (bf16) ----
                if model_mode or "collective" in ablate:
                    nc.sync.dma_start(y_full[0:nlocp, :], y_loc[:, :])
                else:
                    nc.gpsimd.collective_compute(
                        "AllGather", mybir.AluOpType.bypass,
                        replica_groups=[list(range(ncores))],
                        ins=[y_loc.opt()], outs=[y_full.opt()])
                if debug:
                    nc.sync.dma_start(dbg[f"y_full_d{l}"][:, :],
                                      y_full[:, :])

                # ---- aggregation phase ----
                for si, grp in enumerate(sw_groups):
                    c0 = sw_col_start[si]
                    sw_cols = sw_col_start[si + 1] - c0
                    g_t = gpool.tile([P, max_sw_cols, GCH], dt.bfloat16,
                                     tag="g_t")
                    if "gather" in ablate:
                        nc.vector.memset(g_t[:], 0.0)
                    i_t = ipool.tile([P, max_sw_cols * 8], dt.int16,
                                     tag="i_t")
                    nc.sync.dma_start(i_t[:, :sw_cols * 8],
                                      idx16_in[:, c0 * 8:(c0 + sw_cols) * 8])
                    d_t = dpool2.tile([P, max_sw_cols, nsw], dt.bfloat16,
                                      tag="d_t")
                    nc.sync.dma_start(d_t[:, :sw_cols, :],
                                      dstw_in[:, c0:c0 + sw_cols, :])
                    for (r, rc0, rblk) in runs[si]:
                        if rblk == 0 or "gather" in ablate:
                            continue
                        lc = rc0 - c0
                        # split each run across the two SWDGE queues:
                        # descriptor generation runs on both Q7 contexts
                        # concurrently (2 queues measured ~2.4x vs 1)
                        halves = ([(0, rblk)] if rblk == 1 else
                                  [(0, rblk // 2), (rblk // 2, rblk)])
                        for q, (b0, b1) in enumerate(halves):
                            nb = b1 - b0
                            nc.gpsimd.dma_gather(
                                out_ap=g_t[:, lc + b0:lc + b1, :],
                                in_ap=y_full[r * rsize:(r + 1) * rsize, :],
                                idxs_ap=i_t[:, (lc + b0) * 8:(lc + b1) * 8],
                                num_idxs=nb * P, num_idxs_reg=nb * P,
                                elem_size=GCH, single_packet=False,
                                queue_num=(q + r) % 2)
                    if "agg" in ablate:
                        continue
                    for w in grp:
                        wi = w - grp[0]
                        agg = pa_pool.tile([P, GCH], dt.float32, tag="agg")
                        nb_w = sum(b1 - b0 for (b0, b1) in spans[w])
                        done = 0
                        for r in range(NRANGE):
                            b0, b1 = spans[w][r]
                            nb = b1 - b0
                            if nb == 0:
                                continue
                            lc = b0 - c0
                            s_t = spool.tile([P, max_span, P],
                                             dt.bfloat16, tag="s_t")
                            nc.vector.tensor_tensor(
                                out=s_t[:, :nb, :],
                                in0=iota_sb[:, :, :].to_broadcast(
                                    [P, nb, P]),
                                in1=d_t[:, lc:lc + nb, wi:wi + 1]
                                .to_broadcast([P, nb, P]),
                                op=OP.is_equal)
                            for k in range(nb):
                                nc.tensor.matmul(
                                    agg[:], lhsT=s_t[:, k, :],
                                    rhs=g_t[:, lc + k, :],
                                    start=(done == 0),
                                    stop=(done == nb_w - 1))
                                done += 1
                        if l < 2:
                            t_t = epool.tile([P, dout], dt.bfloat16,
                                             tag="t_t")
                            nc.scalar.activation(t_t[:], agg[:, :dout],
                                                 AF.Copy,
                                                 scale=invd_sb[:, w:w + 1])
                            o_t = epool.tile([P, dout], dt.bfloat16,
                                             tag="o_t")
                            nc.vector.tensor_tensor(out=o_t[:], in0=t_t[:],
                                                    in1=r_res[:, w, :dout],
                                                    op=OP.add)
                            nc.vector.tensor_scalar_max(
                                h_sb[l % 2][:, w, :dout], o_t[:], 0.0)
                            if "dense" not in ablate and fuse_dense:
                                dense_w(l + 1, w,
                                        h_sb[l % 2][:, w, :dout])
                        else:
                            t_t = epool.tile([P, dout], dt.float32,
                                             tag="t_t")
                            nc.scalar.activation(t_t[:], agg[:, :dout],
                                                 AF.Copy,
                                                 scale=invd_sb[:, w:w + 1])
                            o_t = epool.tile([P, dout], dt.float32,
                                             tag="o_t")
                            nc.vector.tensor_tensor(out=o_t[:], in0=t_t[:],
                                                    in1=r_res[:, w, :dout],
                                                    op=OP.add)
                            nc.sync.dma_start(h_out[w * P:(w + 1) * P, :],
                                              o_t[:])


    nc.compile()
    return nc


def _preprocess(x, src, dst, ncores=NCORES, nloc=NLOC, nw=NW, nlocp=NLOCP,
                nsw=NSW):
    """Pack per-core edge/index arrays grouped by (super-window, src range).

    Edges are packed edge-granularly inside each (sw, range) run using
    SPMD-uniform per-(window, range) segment sizes (max over cores), so
    only the run total is padded to whole 128-slot blocks. A block may
    hold edges of several windows; the per-window one-hot selects them
    via per-window dstw planes (-1 for foreign slots).

    Returns (per_core input dicts, layout dict for _build_program).
    """
    bf16 = ml_dtypes.bfloat16
    nfullp = ncores * nlocp
    rsize = nfullp // NRANGE

    order = np.argsort(dst, kind="stable")
    src_s = src[order].astype(np.int64)
    dst_s = dst[order].astype(np.int64)
    bounds = np.searchsorted(dst_s, np.arange(ncores + 1) * nloc)

    cores = []
    cnts = np.zeros((ncores, nw, NRANGE), np.int64)
    for c in range(ncores):
        lo, hi = bounds[c], bounds[c + 1]
        s = src_s[lo:hi]
        lcl = dst_s[lo:hi] - c * nloc
        w = lcl // P
        sowner = s // nloc
        s_pad = sowner * nlocp + (s - sowner * nloc)
        rix = s_pad // rsize
        key = w * NRANGE + rix
        o2 = np.lexsort((s_pad, key))
        s_pad, lcl, w, rix, key = (s_pad[o2], lcl[o2], w[o2], rix[o2],
                                   key[o2])
        cnts[c] = np.bincount(key, minlength=nw * NRANGE)\
            .reshape(nw, NRANGE)
        cores.append((s_pad, lcl, w, rix, key))

    # SPMD-uniform segment sizes: max edge count over cores per (w, r)
    seg = cnts.max(axis=0).astype(np.int64)        # [nw, NRANGE]
    seg[:, 0] = np.maximum(seg[:, 0], 1)           # keep every window alive

    sw_groups = _sw_groups(nw, nsw)
    seg_start = np.zeros((nw, NRANGE), np.int64)   # slot offset within run
    run_col = np.zeros((len(sw_groups), NRANGE), np.int64)
    runs = []
    spans = [[None] * NRANGE for _ in range(nw)]   # (b0, b1) global cols
    sw_col_start = [0]
    gc = 0
    for si, grp in enumerate(sw_groups):
        sw_runs = []
        for r in range(NRANGE):
            tot = 0
            for w in grp:
                seg_start[w, r] = tot
                tot += seg[w, r]
            nb = (tot + P - 1) // P
            run_col[si, r] = gc
            for w in grp:
                s0, s1 = seg_start[w, r], seg_start[w, r] + seg[w, r]
                if s1 > s0:
                    spans[w][r] = (int(gc + s0 // P),
                                   int(gc + (s1 + P - 1) // P))
                else:
                    spans[w][r] = (int(gc), int(gc))
            sw_runs.append((r, int(gc), int(nb)))
            gc += nb
        runs.append(sw_runs)
        sw_col_start.append(int(gc))
    total_cols = int(gc)
    max_sw_cols = max(sw_col_start[i + 1] - sw_col_start[i]
                      for i in range(len(sw_groups)))
    max_span = max(b1 - b0 for row in spans for (b0, b1) in row)
    layout = {
        "seg": seg.tolist(),
        "runs": runs,
        "spans": spans,
        "sw_groups": sw_groups,
        "sw_col_start": sw_col_start,
        "total_cols": total_cols,
        "max_sw_cols": int(max_sw_cols),
        "max_span": int(max_span),
    }

    per_core = []
    for c in range(ncores):
        s_pad, lcl, w, rix, key = cores[c]
        cnt = cnts[c]
        starts = np.zeros(nw * NRANGE, np.int64)
        starts[1:] = np.cumsum(cnt.ravel())[:-1]
        j = np.arange(len(lcl)) - starts[key]
        sw = w // nsw
        slot_in_run = seg_start[w, rix] + j
        rcol = run_col[sw, rix]
        col = rcol + slot_in_run // P
        pp = slot_in_run % P
        # per-window dstw planes: plane wi holds the dst offset within
        # window (sw*nsw + wi) for that window's slots, -1 elsewhere
        dstw = np.full((P, total_cols, nsw), -1.0, np.float32)
        dstw[pp, col, w % nsw] = (lcl % P).astype(np.float32)
        i16col = rcol * 8 + slot_in_run // 16
        i16row = slot_in_run % 16
        idx16 = np.zeros((16, total_cols * 8), np.int16)
        idx16[i16row, i16col] = (s_pad - rix * rsize).astype(np.int16)
        idx16 = np.tile(idx16, (8, 1))
        deg = np.bincount(lcl, minlength=nlocp).astype(np.float32)
        invd = (1.0 / np.maximum(deg, 1.0)).reshape(nw, P).T.copy()
        x_pad = np.zeros((nlocp, x.shape[1]), np.float32)
        x_pad[:nloc] = x[c * nloc:(c + 1) * nloc]
        per_core.append({
            "x_local": x_pad,
            "idx16": idx16,
            "dstw": dstw.astype(bf16),
            "invd": invd.astype(np.float32),
        })
    return per_core, layout


def _run_pjrt(nc, in_maps, n_cores, bench_iters=0):
    """Execute the Bass program on the NeuronCores via PJRT/axon.

    Mirrors concourse.bass2jax.run_bass_via_pjrt, with an optional timing
    loop: inputs are pre-placed on device so repeated calls measure
    execute time (plus dispatch overhead) rather than host transfers.
    Returns (per_core_results, best_ns or None).
    """
    import time
    import jax
    import concourse.mybir as mybir
    from concourse.bass2jax import (_bass_exec_p, install_neuronx_cc_hook,
                                    partition_id_tensor)
    from jax.experimental.shard_map import shard_map
    from jax.sharding import Mesh, NamedSharding, PartitionSpec

    install_neuronx_cc_hook()

    partition_name = (nc.partition_id_tensor.name
                      if nc.partition_id_tensor else None)
    in_names, out_names, out_avals, zero_outs = [], [], [], []
    for alloc in nc.m.functions[0].allocations:
        if not isinstance(alloc, mybir.MemoryLocationSet):
            continue
        name = alloc.memorylocations[0].name
        if alloc.kind == "ExternalInput":
            if name != partition_name:
                in_names.append(name)
        elif alloc.kind == "ExternalOutput":
            shape = tuple(alloc.tensor_shape)
            dtype = mybir.dt.np(alloc.dtype)
            out_names.append(name)
            out_avals.append(jax.core.ShapedArray(shape, dtype))
            zero_outs.append(np.zeros(shape, dtype))
    n_params = len(in_names)
    n_outs = len(out_avals)
    in_names.extend(out_names)
    if partition_name is not None:
        in_names.append(partition_name)

    def _body(*args):
        operands = list(args)
        if partition_name is not None:
            operands.append(partition_id_tensor())
        return tuple(_bass_exec_p.bind(
            *operands,
            out_avals=tuple(out_avals),
            in_names=tuple(in_names),
            out_names=tuple(out_names),
            lowering_input_output_aliases=(),
            sim_require_finite=True,
            sim_require_nnan=True,
            nc=nc,
        ))

    devices = jax.devices()[:n_cores]
    assert len(devices) >= n_cores, devices
    mesh = Mesh(np.asarray(devices), ("core",))
    in_specs = (PartitionSpec("core"),) * (n_params + n_outs)
    out_specs = (PartitionSpec("core"),) * n_outs
    sharded = jax.jit(
        shard_map(_body, mesh=mesh, in_specs=in_specs, out_specs=out_specs,
                  check_rep=False),
        keep_unused=True)

    per_core = [[np.asarray(m[name]) for name in in_names[:n_params]]
                for m in in_maps]
    concat_in = [np.concatenate([per_core[c][i] for c in range(n_cores)],
                                axis=0) for i in range(n_params)]
    concat_zeros = [np.zeros((n_cores * z.shape[0], *z.shape[1:]), z.dtype)
                    for z in zero_outs]

    sharding = NamedSharding(mesh, PartitionSpec("core"))
    dev_in = [jax.device_put(a, sharding) for a in concat_in]
    dev_zeros = [jax.device_put(z, sharding) for z in concat_zeros]

    out_arrs = sharded(*dev_in, *dev_zeros)
    out_arrs = [np.asarray(o) for o in out_arrs]

    best_ns = None
    if bench_iters:
        # Device-time measurement through the axon tunnel: issue a chain
        # of async dispatches where each call's (unused) zero-output
        # operands come from the previous call's outputs. The device
        # serializes the NEFF executions while dispatch RPCs pipeline
        # (marginal dispatch cost ~0.2 ms/call at depth >=32), so
        # (t_deep - t_shallow) / (n_deep - n_shallow) isolates per-exec
        # device time and cancels the ~72 ms round-trip.
        def chain(n):
            outs = tuple(dev_zeros)
            t0 = time.perf_counter()
            for _ in range(n):
                outs = sharded(*dev_in, *outs)
            for r in outs:
                r.block_until_ready()
            return time.perf_counter() - t0

        n1, n2 = 12, 12 + max(24, 2 * bench_iters)
        chain(4)  # warm
        for _ in range(3):
            t1 = chain(n1)
            t2 = chain(n2)
            dt_ns = (t2 - t1) / (n2 - n1) * 1e9
            best_ns = dt_ns if best_ns is None else min(best_ns, dt_ns)

    results = [
        {name: out_arrs[i].reshape(n_cores, *out_avals[i].shape)[c]
         for i, name in enumerate(out_names)}
        for c in range(n_cores)
    ]
    return results, best_ns


def kernel(x, edge_index, Wl0, Wr0, b0, Wl1, Wr1, b1, Wl2, Wr2, b2):
    global LAST_EXEC_TIME_NS, LAST_RESULTS

    bf16 = ml_dtypes.bfloat16
    x = np.ascontiguousarray(np.asarray(x, np.float32))
    ei = np.asarray(edge_index)
    src = ei[0].astype(np.int64)
    dst = ei[1].astype(np.int64)

    per_core, layout = _preprocess(x, src, dst)

    Ws = [(np.asarray(Wl0, np.float32), np.asarray(Wr0, np.float32),
           np.asarray(b0, np.float32)),
          (np.asarray(Wl1, np.float32), np.asarray(Wr1, np.float32),
           np.asarray(b1, np.float32)),
          (np.asarray(Wl2, np.float32), np.asarray(Wr2, np.float32),
           np.asarray(b2, np.float32))]
    shared = {}
    for l, (Wl, Wr, b) in enumerate(Ws):
        shared[f"wcat{l}"] = np.ascontiguousarray(
            np.concatenate([Wl, Wr], axis=1).astype(bf16))
        shared[f"bbc{l}"] = np.ascontiguousarray(
            np.tile(b[None, :], (P, 1)).astype(np.float32))
    shared["iota"] = np.tile(np.arange(P, dtype=np.float32)[None, None, :],
                             (P, 1, 1)).astype(bf16)

    in_maps = [{**pc, **shared} for pc in per_core]
    for m in in_maps:
        m["x_local"] = m["x_local"].astype(bf16)

    nc = _build_program(layout)
    bench_iters = int(os.environ.get("GSAGE_BENCH_ITERS", "8"))
    results, best_ns = _run_pjrt(nc, in_maps, NCORES,
                                 bench_iters=bench_iters)
    LAST_EXEC_TIME_NS = best_ns
    LAST_RESULTS = results

    out = np.empty((N_NODES, OUT_CH), np.float32)
    for c in range(NCORES):
        out[c * NLOC:(c + 1) * NLOC] = results[c]["h_out"][:NLOC]
    return out



# revision 39
# speedup vs baseline: 1.0681x; 1.0063x over previous
"""GraphSAGE (3-layer, mean aggregation) on 8 Trainium2 NeuronCores.

Strategy (1D graph partitioning, nodes sharded by row across 8 cores):
  - Core c owns nodes [c*12500, (c+1)*12500); edges partitioned by dst.
  - Per layer l:  Y = h_local @ Wl  (dense, fp32 PE)  -> stored bf16
                  AllGather Y -> Y_full (bf16, replicated per core)
                  gather Y_full[src] rows for this core's edges with
                  dma_gather (int16 indices, 4 source ranges of 25088 rows),
                  then segment-sum by dst via one-hot selection matmuls
                  accumulated in PSUM (fp32)
                  h_new = relu(seg_sum * inv_deg + h_local @ Wr + b)
  - Segment-sum-by-matmul: for a block of 128 edges, S[e, j] = (dst_off[e]==j)
    built on DVE via is_equal against an iota row; PSUM accumulates
    S^T @ G over the window's blocks.

Edges are grouped host-side by (dst-window, src-range); each (window, range)
run is padded to whole 128-edge blocks with slot index 0 (gathers a garbage
row that the all-zero one-hot column ignores). Block counts are maxed across
cores so all 8 cores run one SPMD program.
"""

import os

import numpy as np
import ml_dtypes

P = 128
NCORES = 8
N_NODES = 100000
NLOC = N_NODES // NCORES            # 12500 nodes per core
NW = (NLOC + P - 1) // P            # 98 dst windows per core
NLOCP = NW * P                      # 12544 (padded local nodes)
NFULLP = NCORES * NLOCP             # 100352 (padded global rows)
NRANGE = 4
RSIZE = NFULLP // NRANGE            # 25088 rows per gather range (int16-safe)
NSW = 8                             # windows per super-window (gather batch)
DIMS = [(128, 128), (128, 128), (128, 64)]
GCH = 128                           # gathered feature columns (Y3 zero-padded)
OUT_CH = 64

LAST_EXEC_TIME_NS = None
LAST_RESULTS = None


def _sw_groups(nw, nsw):
    return [list(range(a, min(a + nsw, nw))) for a in range(0, nw, nsw)]


def _build_program(layout, nw=NW, nlocp=NLOCP, nfullp=NFULLP, ncores=NCORES,
                   dims=DIMS, debug=False, model_mode=False, ablate=(),
                   fuse_dense=True):
    """layout: dict with nblk [nw][4], plus derived column/idx offsets.

    ablate: subset of {"gather", "agg", "dense", "collective"} — skip that
    phase (produces wrong results; for timing attribution only).
    """
    import concourse.bacc as bacc
    import concourse.bass as bass
    import concourse.mybir as mybir
    import concourse.tile as tile
    from concourse.masks import make_identity

    dt = mybir.dt
    AF = mybir.ActivationFunctionType
    OP = mybir.AluOpType
    out_ch = dims[-1][1]
    rsize = nfullp // NRANGE

    runs = layout["runs"]              # per sw: list of (r, col_start, nblks)
    spans = layout["spans"]            # [nw][NRANGE] (b0, b1) global cols
    sw_groups = layout["sw_groups"]
    total_cols = layout["total_cols"]
    max_sw_cols = layout["max_sw_cols"]
    max_span = layout["max_span"]
    sw_col_start = layout["sw_col_start"]
    nsw = len(sw_groups[0])

    nc = bacc.Bacc("TRN2", target_bir_lowering=False, debug=False,
                   num_devices=ncores, num_swdge_queues=2)

    x_in = nc.dram_tensor("x_local", [nlocp, dims[0][0]], dt.bfloat16,
                          kind="ExternalInput")
    wcat_in = [nc.dram_tensor(f"wcat{l}", [dims[l][0], 2 * dims[l][1]],
                              dt.bfloat16, kind="ExternalInput")
               for l in range(3)]
    bbc_in = [nc.dram_tensor(f"bbc{l}", [P, dims[l][1]], dt.float32,
                             kind="ExternalInput") for l in range(3)]
    idx16_in = nc.dram_tensor("idx16", [P, total_cols * 8], dt.int16,
                              kind="ExternalInput")
    dstw_in = nc.dram_tensor("dstw", [P, total_cols, nsw], dt.bfloat16,
                             kind="ExternalInput")
    invd_in = nc.dram_tensor("invd", [P, nw], dt.float32,
                             kind="ExternalInput")
    iota_in = nc.dram_tensor("iota", [P, 1, P], dt.bfloat16,
                             kind="ExternalInput")
    h_out = nc.dram_tensor("h_out", [nlocp, out_ch], dt.float32,
                           kind="ExternalOutput")
    dbg = {}
    if debug:
        for l in range(3):
            dbg[f"y_full_d{l}"] = nc.dram_tensor(
                f"y_full_d{l}", [nfullp, GCH], dt.bfloat16,
                kind="ExternalOutput")

    with tile.TileContext(nc) as tc:
        with (
            tc.tile_pool(name="const", bufs=1) as cpool,
            tc.tile_pool(name="dram", bufs=1, space="DRAM") as dpool,
            tc.tile_pool(name="hload", bufs=4) as hpool,
            tc.tile_pool(name="htr", bufs=4) as htpool,
            tc.tile_pool(name="yt", bufs=4) as ypool,
            tc.tile_pool(name="gat", bufs=2) as gpool,
            tc.tile_pool(name="idx", bufs=3) as ipool,
            tc.tile_pool(name="dwp", bufs=3) as dpool2,
            tc.tile_pool(name="sel", bufs=8) as spool,
            tc.tile_pool(name="epi", bufs=8) as epool,
            tc.tile_pool(name="pst", bufs=2, space="PSUM") as pt_pool,
            tc.tile_pool(name="psm", bufs=2, space="PSUM") as pmm_pool,
            tc.tile_pool(name="psa", bufs=4, space="PSUM") as pa_pool,
        ):
            ident = cpool.tile([P, P], dt.bfloat16)
            make_identity(nc, ident[:])
            iota_sb = cpool.tile([P, 1, P], dt.bfloat16)
            nc.sync.dma_start(iota_sb[:], iota_in[:, :, :])
            invd_sb = cpool.tile([P, nw], dt.float32)
            nc.sync.dma_start(invd_sb[:], invd_in[:, :])
            wc_sb = []
            bb_sb = []
            for l in range(3):
                w_t = cpool.tile([dims[l][0], 2 * dims[l][1]], dt.bfloat16,
                                 name=f"wc{l}")
                nc.sync.dma_start(w_t[:], wcat_in[l][:, :])
                wc_sb.append(w_t)
                b_t = cpool.tile([P, dims[l][1]], dt.float32, name=f"bb{l}")
                nc.sync.dma_start(b_t[:], bbc_in[l][:, :])
                bb_sb.append(b_t)
            r_res = cpool.tile([P, nw, dims[0][1]], dt.bfloat16)
            if "dense" in ablate:
                nc.vector.memset(r_res[:], 0.0)
            # h for layers 1/2 lives in SBUF (bf16), window-sliced: the
            # next layer's dense phase reads it without any DRAM round
            # trip and can overlap the previous layer's aggregation.
            h_sb = [cpool.tile([P, nw, dims[0][1]], dt.bfloat16,
                               name=f"hsb{i}") for i in range(2)]
            if "agg" in ablate:
                for t in h_sb:
                    nc.vector.memset(t[:], 0.0)

            y_locs = [dpool.tile([nlocp, GCH], dt.bfloat16,
                                 name=f"y_loc{l}") for l in range(3)]
            y_fulls = [dpool.tile([nfullp, GCH], dt.bfloat16,
                                  addr_space="Shared", name=f"y_full{l}")
                       for l in range(3)]

            def dense_w(l, i, h_ap):
                # Y_w = h_w @ Wl (-> bf16 y_loc), r_res_w = h_w @ Wr + b
                din, dout = dims[l]
                t_ps = pt_pool.tile([P, P], dt.bfloat16, tag="t_ps")
                nc.tensor.transpose(t_ps[:din, :], h_ap, ident[:])
                hT = htpool.tile([P, P], dt.bfloat16, tag="hT")
                nc.vector.tensor_copy(hT[:din, :], t_ps[:din, :])
                mm = pmm_pool.tile([P, 2 * dout], dt.float32, tag="mm")
                nc.tensor.matmul(mm[:], lhsT=hT[:din, :],
                                 rhs=wc_sb[l][:, :], start=True, stop=True)
                y_t = ypool.tile([P, GCH], dt.bfloat16, tag="y_t")
                nc.scalar.activation(y_t[:, :dout], mm[:, :dout], AF.Copy)
                if dout < GCH:
                    nc.vector.memset(y_t[:, dout:], 0.0)
                nc.sync.dma_start(y_locs[l][i * P:(i + 1) * P, :], y_t[:])
                nc.vector.tensor_tensor(out=r_res[:, i, :dout],
                                        in0=mm[:, dout:2 * dout],
                                        in1=bb_sb[l][:, :], op=OP.add)

            for l in range(3):
                din, dout = dims[l]
                y_loc = y_locs[l]
                y_full = y_fulls[l]

                # ---- dense phase: layer 0 from x; layers 1/2 are emitted
                # inside the previous layer's epilogue (window-fused) so
                # they overlap that layer's gathers/aggregation ----
                if "dense" not in ablate and l == 0:
                    for i in range(nw):
                        h_t = hpool.tile([P, din], dt.bfloat16, tag="h_t")
                        nc.sync.dma_start(h_t[:],
                                          x_in[i * P:(i + 1) * P, :])
                        dense_w(0, i, h_t[:])
                if "dense" not in ablate and l > 0 and not fuse_dense:
                    for i in range(nw):
                        dense_w(l, i, h_sb[(l - 1) % 2][:, i, :din])

                # ---- AllGather Y (bf16) ----
                if model_mode or "collective" in ablate:
                    nc.sync.dma_start(y_full[0:nlocp, :], y_loc[:, :])
                else:
                    nc.gpsimd.collective_compute(
                        "AllGather", mybir.AluOpType.bypass,
                        replica_groups=[list(range(ncores))],
                        ins=[y_loc.opt()], outs=[y_full.opt()])
                if debug:
                    nc.sync.dma_start(dbg[f"y_full_d{l}"][:, :],
                                      y_full[:, :])

                # ---- aggregation phase ----
                for si, grp in enumerate(sw_groups):
                    c0 = sw_col_start[si]
                    sw_cols = sw_col_start[si + 1] - c0
                    g_t = gpool.tile([P, max_sw_cols, GCH], dt.bfloat16,
                                     tag="g_t")
                    if "gather" in ablate:
                        nc.vector.memset(g_t[:], 0.0)
                    i_t = ipool.tile([P, max_sw_cols * 8], dt.int16,
                                     tag="i_t")
                    nc.sync.dma_start(i_t[:, :sw_cols * 8],
                                      idx16_in[:, c0 * 8:(c0 + sw_cols) * 8])
                    d_t = dpool2.tile([P, max_sw_cols, nsw], dt.bfloat16,
                                      tag="d_t")
                    nc.sync.dma_start(d_t[:, :sw_cols, :],
                                      dstw_in[:, c0:c0 + sw_cols, :])
                    for (r, rc0, rblk) in runs[si]:
                        if rblk == 0 or "gather" in ablate:
                            continue
                        lc = rc0 - c0
                        # split each run across the two SWDGE queues:
                        # descriptor generation runs on both Q7 contexts
                        # concurrently (2 queues measured ~2.4x vs 1)
                        halves = ([(0, rblk)] if rblk == 1 else
                                  [(0, rblk // 2), (rblk // 2, rblk)])
                        for q, (b0, b1) in enumerate(halves):
                            nb = b1 - b0
                            nc.gpsimd.dma_gather(
                                out_ap=g_t[:, lc + b0:lc + b1, :],
                                in_ap=y_full[r * rsize:(r + 1) * rsize, :],
                                idxs_ap=i_t[:, (lc + b0) * 8:(lc + b1) * 8],
                                num_idxs=nb * P, num_idxs_reg=nb * P,
                                elem_size=GCH, single_packet=False,
                                queue_num=(q + r) % 2)
                    if "agg" in ablate:
                        continue
                    for w in grp:
                        wi = w - grp[0]
                        agg = pa_pool.tile([P, GCH], dt.float32, tag="agg")
                        nb_w = sum(b1 - b0 for (b0, b1) in spans[w])
                        done = 0
                        for r in range(NRANGE):
                            b0, b1 = spans[w][r]
                            nb = b1 - b0
                            if nb == 0:
                                continue
                            lc = b0 - c0
                            s_t = spool.tile([P, max_span, P],
                                             dt.bfloat16, tag="s_t")
                            nc.vector.tensor_tensor(
                                out=s_t[:, :nb, :],
                                in0=iota_sb[:, :, :].to_broadcast(
                                    [P, nb, P]),
                                in1=d_t[:, lc:lc + nb, wi:wi + 1]
                                .to_broadcast([P, nb, P]),
                                op=OP.is_equal)
                            for k in range(nb):
                                nc.tensor.matmul(
                                    agg[:], lhsT=s_t[:, k, :],
                                    rhs=g_t[:, lc + k, :],
                                    start=(done == 0),
                                    stop=(done == nb_w - 1))
                                done += 1
                        if l < 2:
                            t_t = epool.tile([P, dout], dt.bfloat16,
                                             tag="t_t")
                            nc.scalar.activation(t_t[:], agg[:, :dout],
                                                 AF.Copy,
                                                 scale=invd_sb[:, w:w + 1])
                            o_t = epool.tile([P, dout], dt.bfloat16,
                                             tag="o_t")
                            nc.vector.tensor_tensor(out=o_t[:], in0=t_t[:],
                                                    in1=r_res[:, w, :dout],
                                                    op=OP.add)
                            nc.vector.tensor_scalar_max(
                                h_sb[l % 2][:, w, :dout], o_t[:], 0.0)
                            if "dense" not in ablate and fuse_dense:
                                dense_w(l + 1, w,
                                        h_sb[l % 2][:, w, :dout])
                        else:
                            t_t = epool.tile([P, dout], dt.float32,
                                             tag="t_t")
                            nc.scalar.activation(t_t[:], agg[:, :dout],
                                                 AF.Copy,
                                                 scale=invd_sb[:, w:w + 1])
                            o_t = epool.tile([P, dout], dt.float32,
                                             tag="o_t")
                            nc.vector.tensor_tensor(out=o_t[:], in0=t_t[:],
                                                    in1=r_res[:, w, :dout],
                                                    op=OP.add)
                            nc.sync.dma_start(h_out[w * P:(w + 1) * P, :],
                                              o_t[:])


    nc.compile()
    return nc


def _preprocess(x, src, dst, ncores=NCORES, nloc=NLOC, nw=NW, nlocp=NLOCP,
                nsw=NSW):
    """Pack per-core edge/index arrays grouped by (super-window, src range).

    Edges are packed edge-granularly inside each (sw, range) run using
    SPMD-uniform per-(window, range) segment sizes (max over cores), so
    only the run total is padded to whole 128-slot blocks. A block may
    hold edges of several windows; the per-window one-hot selects them
    via per-window dstw planes (-1 for foreign slots).

    Returns (per_core input dicts, layout dict for _build_program).
    """
    bf16 = ml_dtypes.bfloat16
    nfullp = ncores * nlocp
    rsize = nfullp // NRANGE

    order = np.argsort(dst, kind="stable")
    src_s = src[order].astype(np.int64)
    dst_s = dst[order].astype(np.int64)
    bounds = np.searchsorted(dst_s, np.arange(ncores + 1) * nloc)

    cores = []
    cnts = np.zeros((ncores, nw, NRANGE), np.int64)
    for c in range(ncores):
        lo, hi = bounds[c], bounds[c + 1]
        s = src_s[lo:hi]
        lcl = dst_s[lo:hi] - c * nloc
        w = lcl // P
        sowner = s // nloc
        s_pad = sowner * nlocp + (s - sowner * nloc)
        rix = s_pad // rsize
        key = w * NRANGE + rix
        o2 = np.lexsort((s_pad, key))
        s_pad, lcl, w, rix, key = (s_pad[o2], lcl[o2], w[o2], rix[o2],
                                   key[o2])
        cnts[c] = np.bincount(key, minlength=nw * NRANGE)\
            .reshape(nw, NRANGE)
        cores.append((s_pad, lcl, w, rix, key))

    # SPMD-uniform segment sizes: max edge count over cores per (w, r)
    seg = cnts.max(axis=0).astype(np.int64)        # [nw, NRANGE]
    seg[:, 0] = np.maximum(seg[:, 0], 1)           # keep every window alive

    sw_groups = _sw_groups(nw, nsw)
    seg_start = np.zeros((nw, NRANGE), np.int64)   # slot offset within run
    run_col = np.zeros((len(sw_groups), NRANGE), np.int64)
    runs = []
    spans = [[None] * NRANGE for _ in range(nw)]   # (b0, b1) global cols
    sw_col_start = [0]
    gc = 0
    for si, grp in enumerate(sw_groups):
        sw_runs = []
        for r in range(NRANGE):
            tot = 0
            for w in grp:
                seg_start[w, r] = tot
                tot += seg[w, r]
            nb = (tot + P - 1) // P
            run_col[si, r] = gc
            for w in grp:
                s0, s1 = seg_start[w, r], seg_start[w, r] + seg[w, r]
                if s1 > s0:
                    spans[w][r] = (int(gc + s0 // P),
                                   int(gc + (s1 + P - 1) // P))
                else:
                    spans[w][r] = (int(gc), int(gc))
            sw_runs.append((r, int(gc), int(nb)))
            gc += nb
        runs.append(sw_runs)
        sw_col_start.append(int(gc))
    total_cols = int(gc)
    max_sw_cols = max(sw_col_start[i + 1] - sw_col_start[i]
                      for i in range(len(sw_groups)))
    max_span = max(b1 - b0 for row in spans for (b0, b1) in row)
    layout = {
        "seg": seg.tolist(),
        "runs": runs,
        "spans": spans,
        "sw_groups": sw_groups,
        "sw_col_start": sw_col_start,
        "total_cols": total_cols,
        "max_sw_cols": int(max_sw_cols),
        "max_span": int(max_span),
    }

    per_core = []
    for c in range(ncores):
        s_pad, lcl, w, rix, key = cores[c]
        cnt = cnts[c]
        starts = np.zeros(nw * NRANGE, np.int64)
        starts[1:] = np.cumsum(cnt.ravel())[:-1]
        j = np.arange(len(lcl)) - starts[key]
        sw = w // nsw
        slot_in_run = seg_start[w, rix] + j
        rcol = run_col[sw, rix]
        col = rcol + slot_in_run // P
        pp = slot_in_run % P
        # per-window dstw planes: plane wi holds the dst offset within
        # window (sw*nsw + wi) for that window's slots, -1 elsewhere
        dstw = np.full((P, total_cols, nsw), -1.0, np.float32)
        dstw[pp, col, w % nsw] = (lcl % P).astype(np.float32)
        i16col = rcol * 8 + slot_in_run // 16
        i16row = slot_in_run % 16
        idx16 = np.zeros((16, total_cols * 8), np.int16)
        idx16[i16row, i16col] = (s_pad - rix * rsize).astype(np.int16)
        idx16 = np.tile(idx16, (8, 1))
        deg = np.bincount(lcl, minlength=nlocp).astype(np.float32)
        invd = (1.0 / np.maximum(deg, 1.0)).reshape(nw, P).T.copy()
        x_pad = np.zeros((nlocp, x.shape[1]), np.float32)
        x_pad[:nloc] = x[c * nloc:(c + 1) * nloc]
        per_core.append({
            "x_local": x_pad,
            "idx16": idx16,
            "dstw": dstw.astype(bf16),
            "invd": invd.astype(np.float32),
        })
    return per_core, layout


def _run_pjrt(nc, in_maps, n_cores, bench_iters=0):
    """Execute the Bass program on the NeuronCores via PJRT/axon.

    Mirrors concourse.bass2jax.run_bass_via_pjrt, with an optional timing
    loop: inputs are pre-placed on device so repeated calls measure
    execute time (plus dispatch overhead) rather than host transfers.
    Returns (per_core_results, best_ns or None).
    """
    import time
    import jax
    import concourse.mybir as mybir
    from concourse.bass2jax import (_bass_exec_p, install_neuronx_cc_hook,
                                    partition_id_tensor)
    from jax.experimental.shard_map import shard_map
    from jax.sharding import Mesh, NamedSharding, PartitionSpec

    install_neuronx_cc_hook()

    partition_name = (nc.partition_id_tensor.name
                      if nc.partition_id_tensor else None)
    in_names, out_names, out_avals, zero_outs = [], [], [], []
    for alloc in nc.m.functions[0].allocations:
        if not isinstance(alloc, mybir.MemoryLocationSet):
            continue
        name = alloc.memorylocations[0].name
        if alloc.kind == "ExternalInput":
            if name != partition_name:
                in_names.append(name)
        elif alloc.kind == "ExternalOutput":
            shape = tuple(alloc.tensor_shape)
            dtype = mybir.dt.np(alloc.dtype)
            out_names.append(name)
            out_avals.append(jax.core.ShapedArray(shape, dtype))
            zero_outs.append(np.zeros(shape, dtype))
    n_params = len(in_names)
    n_outs = len(out_avals)
    in_names.extend(out_names)
    if partition_name is not None:
        in_names.append(partition_name)

    def _body(*args):
        operands = list(args)
        if partition_name is not None:
            operands.append(partition_id_tensor())
        return tuple(_bass_exec_p.bind(
            *operands,
            out_avals=tuple(out_avals),
            in_names=tuple(in_names),
            out_names=tuple(out_names),
            lowering_input_output_aliases=(),
            sim_require_finite=True,
            sim_require_nnan=True,
            nc=nc,
        ))

    devices = jax.devices()[:n_cores]
    assert len(devices) >= n_cores, devices
    mesh = Mesh(np.asarray(devices), ("core",))
    in_specs = (PartitionSpec("core"),) * (n_params + n_outs)
    out_specs = (PartitionSpec("core"),) * n_outs
    sharded = jax.jit(
        shard_map(_body, mesh=mesh, in_specs=in_specs, out_specs=out_specs,
                  check_rep=False),
        keep_unused=True)

    per_core = [[np.asarray(m[name]) for name in in_names[:n_params]]
                for m in in_maps]
    concat_in = [np.concatenate([per_core[c][i] for c in range(n_cores)],
                                axis=0) for i in range(n_params)]
    concat_zeros = [np.zeros((n_cores * z.shape[0], *z.shape[1:]), z.dtype)
                    for z in zero_outs]

    sharding = NamedSharding(mesh, PartitionSpec("core"))
    dev_in = [jax.device_put(a, sharding) for a in concat_in]
    dev_zeros = [jax.device_put(z, sharding) for z in concat_zeros]

    out_arrs = sharded(*dev_in, *dev_zeros)
    out_arrs = [np.asarray(o) for o in out_arrs]

    best_ns = None
    if bench_iters:
        # Device-time measurement through the axon tunnel: issue a chain
        # of async dispatches where each call's (unused) zero-output
        # operands come from the previous call's outputs. The device
        # serializes the NEFF executions while dispatch RPCs pipeline
        # (marginal dispatch cost ~0.2 ms/call at depth >=32), so
        # (t_deep - t_shallow) / (n_deep - n_shallow) isolates per-exec
        # device time and cancels the ~72 ms round-trip.
        def chain(n):
            outs = tuple(dev_zeros)
            t0 = time.perf_counter()
            for _ in range(n):
                outs = sharded(*dev_in, *outs)
            for r in outs:
                r.block_until_ready()
            return time.perf_counter() - t0

        n1, n2 = 12, 12 + max(24, 2 * bench_iters)
        chain(4)  # warm
        for _ in range(3):
            t1 = chain(n1)
            t2 = chain(n2)
            dt_ns = (t2 - t1) / (n2 - n1) * 1e9
            best_ns = dt_ns if best_ns is None else min(best_ns, dt_ns)

    results = [
        {name: out_arrs[i].reshape(n_cores, *out_avals[i].shape)[c]
         for i, name in enumerate(out_names)}
        for c in range(n_cores)
    ]
    return results, best_ns


def kernel(x, edge_index, Wl0, Wr0, b0, Wl1, Wr1, b1, Wl2, Wr2, b2):
    global LAST_EXEC_TIME_NS, LAST_RESULTS

    bf16 = ml_dtypes.bfloat16
    x = np.ascontiguousarray(np.asarray(x, np.float32))
    ei = np.asarray(edge_index)
    src = ei[0].astype(np.int64)
    dst = ei[1].astype(np.int64)

    per_core, layout = _preprocess(x, src, dst)

    Ws = [(np.asarray(Wl0, np.float32), np.asarray(Wr0, np.float32),
           np.asarray(b0, np.float32)),
          (np.asarray(Wl1, np.float32), np.asarray(Wr1, np.float32),
           np.asarray(b1, np.float32)),
          (np.asarray(Wl2, np.float32), np.asarray(Wr2, np.float32),
           np.asarray(b2, np.float32))]
    shared = {}
    for l, (Wl, Wr, b) in enumerate(Ws):
        shared[f"wcat{l}"] = np.ascontiguousarray(
            np.concatenate([Wl, Wr], axis=1).astype(bf16))
        shared[f"bbc{l}"] = np.ascontiguousarray(
            np.tile(b[None, :], (P, 1)).astype(np.float32))
    shared["iota"] = np.tile(np.arange(P, dtype=np.float32)[None, None, :],
                             (P, 1, 1)).astype(bf16)

    in_maps = [{**pc, **shared} for pc in per_core]
    for m in in_maps:
        m["x_local"] = m["x_local"].astype(bf16)

    nc = _build_program(layout)
    bench_iters = int(os.environ.get("GSAGE_BENCH_ITERS", "8"))
    results, best_ns = _run_pjrt(nc, in_maps, NCORES,
                                 bench_iters=bench_iters)
    LAST_EXEC_TIME_NS = best_ns
    LAST_RESULTS = results

    out = np.empty((N_NODES, OUT_CH), np.float32)
    for c in range(NCORES):
        out[c * NLOC:(c + 1) * NLOC] = results[c]["h_out"][:NLOC]
    return out



# revision 40
# speedup vs baseline: 1.3610x; 1.2742x over previous
"""GraphSAGE (3-layer, mean aggregation) on 8 Trainium2 NeuronCores.

Strategy (1D graph partitioning, nodes sharded by row across 8 cores):
  - Core c owns nodes [c*12500, (c+1)*12500); edges partitioned by dst.
  - Per layer l:  Y = h_local @ Wl  (dense, fp32 PE)  -> stored bf16
                  AllGather Y -> Y_full (bf16, replicated per core)
                  gather Y_full[src] rows for this core's edges with
                  dma_gather (int16 indices, 4 source ranges of 25088 rows),
                  then segment-sum by dst via one-hot selection matmuls
                  accumulated in PSUM (fp32)
                  h_new = relu(seg_sum * inv_deg + h_local @ Wr + b)
  - Segment-sum-by-matmul: for a block of 128 edges, S[e, j] = (dst_off[e]==j)
    built on DVE via is_equal against an iota row; PSUM accumulates
    S^T @ G over the window's blocks.

Edges are grouped host-side by (dst-window, src-range); each (window, range)
run is padded to whole 128-edge blocks with slot index 0 (gathers a garbage
row that the all-zero one-hot column ignores). Block counts are maxed across
cores so all 8 cores run one SPMD program.
"""

import os

import numpy as np
import ml_dtypes

P = 128
NCORES = 8
N_NODES = 100000
NLOC = N_NODES // NCORES            # 12500 nodes per core
NW = (NLOC + P - 1) // P            # 98 dst windows per core
NLOCP = NW * P                      # 12544 (padded local nodes)
NFULLP = NCORES * NLOCP             # 100352 (padded global rows)
NRANGE = 4
RSIZE = NFULLP // NRANGE            # 25088 rows per gather range (int16-safe)
NSW = 8                             # windows per super-window (gather batch)
DIMS = [(128, 128), (128, 128), (128, 64)]
GCH = 128                           # gathered feature columns (Y3 zero-padded)
OUT_CH = 64

LAST_EXEC_TIME_NS = None
LAST_RESULTS = None


def _sw_groups(nw, nsw):
    return [list(range(a, min(a + nsw, nw))) for a in range(0, nw, nsw)]


def _build_program(layout, nw=NW, nlocp=NLOCP, nfullp=NFULLP, ncores=NCORES,
                   dims=DIMS, debug=False, model_mode=False, ablate=(),
                   fuse_dense=True):
    """layout: dict with nblk [nw][4], plus derived column/idx offsets.

    ablate: subset of {"gather", "agg", "dense", "collective"} — skip that
    phase (produces wrong results; for timing attribution only).
    """
    import concourse.bacc as bacc
    import concourse.bass as bass
    import concourse.mybir as mybir
    import concourse.tile as tile
    from concourse.masks import make_identity

    dt = mybir.dt
    AF = mybir.ActivationFunctionType
    OP = mybir.AluOpType
    out_ch = dims[-1][1]
    rsize = nfullp // NRANGE

    runs = layout["runs"]              # per sw: list of (r, col_start, nblks)
    spans = layout["spans"]            # [nw][NRANGE] (b0, b1) global cols
    sw_groups = layout["sw_groups"]
    total_cols = layout["total_cols"]
    max_sw_cols = layout["max_sw_cols"]
    max_span = layout["max_span"]
    sw_col_start = layout["sw_col_start"]
    nsw = len(sw_groups[0])

    nc = bacc.Bacc("TRN2", target_bir_lowering=False, debug=False,
                   num_devices=ncores, num_swdge_queues=3)

    x_in = nc.dram_tensor("x_local", [nlocp, dims[0][0]], dt.bfloat16,
                          kind="ExternalInput")
    wcat_in = [nc.dram_tensor(f"wcat{l}", [dims[l][0], 2 * dims[l][1]],
                              dt.bfloat16, kind="ExternalInput")
               for l in range(3)]
    bbc_in = [nc.dram_tensor(f"bbc{l}", [P, dims[l][1]], dt.float32,
                             kind="ExternalInput") for l in range(3)]
    idx16_in = nc.dram_tensor("idx16", [P, total_cols * 8], dt.int16,
                              kind="ExternalInput")
    dstw_in = nc.dram_tensor("dstw", [P, total_cols, nsw], dt.bfloat16,
                             kind="ExternalInput")
    invd_in = nc.dram_tensor("invd", [P, nw], dt.float32,
                             kind="ExternalInput")
    iota_in = nc.dram_tensor("iota", [P, 1, P], dt.bfloat16,
                             kind="ExternalInput")
    h_out = nc.dram_tensor("h_out", [nlocp, out_ch], dt.float32,
                           kind="ExternalOutput")
    dbg = {}
    if debug:
        for l in range(3):
            dbg[f"y_full_d{l}"] = nc.dram_tensor(
                f"y_full_d{l}", [nfullp, GCH], dt.bfloat16,
                kind="ExternalOutput")

    with tile.TileContext(nc) as tc:
        with (
            tc.tile_pool(name="const", bufs=1) as cpool,
            tc.tile_pool(name="dram", bufs=1, space="DRAM") as dpool,
            tc.tile_pool(name="hload", bufs=4) as hpool,
            tc.tile_pool(name="htr", bufs=4) as htpool,
            tc.tile_pool(name="yt", bufs=4) as ypool,
            tc.tile_pool(name="gat", bufs=2) as gpool,
            tc.tile_pool(name="idx", bufs=3) as ipool,
            tc.tile_pool(name="dwp", bufs=3) as dpool2,
            tc.tile_pool(name="sel", bufs=8) as spool,
            tc.tile_pool(name="epi", bufs=8) as epool,
            tc.tile_pool(name="pst", bufs=2, space="PSUM") as pt_pool,
            tc.tile_pool(name="psm", bufs=2, space="PSUM") as pmm_pool,
            tc.tile_pool(name="psa", bufs=4, space="PSUM") as pa_pool,
        ):
            ident = cpool.tile([P, P], dt.bfloat16)
            make_identity(nc, ident[:])
            iota_sb = cpool.tile([P, 1, P], dt.bfloat16)
            nc.sync.dma_start(iota_sb[:], iota_in[:, :, :])
            invd_sb = cpool.tile([P, nw], dt.float32)
            nc.sync.dma_start(invd_sb[:], invd_in[:, :])
            wc_sb = []
            bb_sb = []
            for l in range(3):
                w_t = cpool.tile([dims[l][0], 2 * dims[l][1]], dt.bfloat16,
                                 name=f"wc{l}")
                nc.sync.dma_start(w_t[:], wcat_in[l][:, :])
                wc_sb.append(w_t)
                b_t = cpool.tile([P, dims[l][1]], dt.float32, name=f"bb{l}")
                nc.sync.dma_start(b_t[:], bbc_in[l][:, :])
                bb_sb.append(b_t)
            r_res = cpool.tile([P, nw, dims[0][1]], dt.bfloat16)
            if "dense" in ablate:
                nc.vector.memset(r_res[:], 0.0)
            # h for layers 1/2 lives in SBUF (bf16), window-sliced: the
            # next layer's dense phase reads it without any DRAM round
            # trip and can overlap the previous layer's aggregation.
            h_sb = [cpool.tile([P, nw, dims[0][1]], dt.bfloat16,
                               name=f"hsb{i}") for i in range(2)]
            if "agg" in ablate:
                for t in h_sb:
                    nc.vector.memset(t[:], 0.0)

            y_locs = [dpool.tile([nlocp, GCH], dt.bfloat16,
                                 name=f"y_loc{l}") for l in range(3)]
            y_fulls = [dpool.tile([nfullp, GCH], dt.bfloat16,
                                  addr_space="Shared", name=f"y_full{l}")
                       for l in range(3)]

            def dense_w(l, i, h_ap):
                # Y_w = h_w @ Wl (-> bf16 y_loc), r_res_w = h_w @ Wr + b
                din, dout = dims[l]
                t_ps = pt_pool.tile([P, P], dt.bfloat16, tag="t_ps")
                nc.tensor.transpose(t_ps[:din, :], h_ap, ident[:])
                hT = htpool.tile([P, P], dt.bfloat16, tag="hT")
                nc.vector.tensor_copy(hT[:din, :], t_ps[:din, :])
                mm = pmm_pool.tile([P, 2 * dout], dt.float32, tag="mm")
                nc.tensor.matmul(mm[:], lhsT=hT[:din, :],
                                 rhs=wc_sb[l][:, :], start=True, stop=True)
                y_t = ypool.tile([P, GCH], dt.bfloat16, tag="y_t")
                nc.scalar.activation(y_t[:, :dout], mm[:, :dout], AF.Copy)
                if dout < GCH:
                    nc.vector.memset(y_t[:, dout:], 0.0)
                nc.sync.dma_start(y_locs[l][i * P:(i + 1) * P, :], y_t[:])
                nc.vector.tensor_tensor(out=r_res[:, i, :dout],
                                        in0=mm[:, dout:2 * dout],
                                        in1=bb_sb[l][:, :], op=OP.add)

            for l in range(3):
                din, dout = dims[l]
                y_loc = y_locs[l]
                y_full = y_fulls[l]

                # ---- dense phase: layer 0 from x; layers 1/2 are emitted
                # inside the previous layer's epilogue (window-fused) so
                # they overlap that layer's gathers/aggregation ----
                if "dense" not in ablate and l == 0:
                    for i in range(nw):
                        h_t = hpool.tile([P, din], dt.bfloat16, tag="h_t")
                        nc.sync.dma_start(h_t[:],
                                          x_in[i * P:(i + 1) * P, :])
                        dense_w(0, i, h_t[:])
                if "dense" not in ablate and l > 0 and not fuse_dense:
                    for i in range(nw):
                        dense_w(l, i, h_sb[(l - 1) % 2][:, i, :din])

                # ---- AllGather Y (bf16) ----
                if model_mode or "collective" in ablate:
                    nc.sync.dma_start(y_full[0:nlocp, :], y_loc[:, :])
                else:
                    nc.gpsimd.collective_compute(
                        "AllGather", mybir.AluOpType.bypass,
                        replica_groups=[list(range(ncores))],
                        ins=[y_loc.opt()], outs=[y_full.opt()])
                if debug:
                    nc.sync.dma_start(dbg[f"y_full_d{l}"][:, :],
                                      y_full[:, :])

                # ---- aggregation phase ----
                for si, grp in enumerate(sw_groups):
                    c0 = sw_col_start[si]
                    sw_cols = sw_col_start[si + 1] - c0
                    g_t = gpool.tile([P, max_sw_cols, GCH], dt.bfloat16,
                                     tag="g_t")
                    if "gather" in ablate:
                        nc.vector.memset(g_t[:], 0.0)
                    i_t = ipool.tile([P, max_sw_cols * 8], dt.int16,
                                     tag="i_t")
                    nc.sync.dma_start(i_t[:, :sw_cols * 8],
                                      idx16_in[:, c0 * 8:(c0 + sw_cols) * 8])
                    d_t = dpool2.tile([P, max_sw_cols, nsw], dt.bfloat16,
                                      tag="d_t")
                    nc.sync.dma_start(d_t[:, :sw_cols, :],
                                      dstw_in[:, c0:c0 + sw_cols, :])
                    for (r, rc0, rblk) in runs[si]:
                        if rblk == 0 or "gather" in ablate:
                            continue
                        lc = rc0 - c0
                        # split each run across the three SWDGE queues:
                        # descriptor generation runs on multiple Q7
                        # contexts concurrently (3 queues ~3x vs 1)
                        halves = ([(0, rblk)] if rblk == 1 else
                                  [(0, rblk // 2), (rblk // 2, rblk)])
                        for q, (b0, b1) in enumerate(halves):
                            nb = b1 - b0
                            nc.gpsimd.dma_gather(
                                out_ap=g_t[:, lc + b0:lc + b1, :],
                                in_ap=y_full[r * rsize:(r + 1) * rsize, :],
                                idxs_ap=i_t[:, (lc + b0) * 8:(lc + b1) * 8],
                                num_idxs=nb * P, num_idxs_reg=nb * P,
                                elem_size=GCH, single_packet=False,
                                queue_num=(2 * r + q) % 3)
                    if "agg" in ablate:
                        continue
                    for w in grp:
                        wi = w - grp[0]
                        agg = pa_pool.tile([P, GCH], dt.float32, tag="agg")
                        nb_w = sum(b1 - b0 for (b0, b1) in spans[w])
                        done = 0
                        for r in range(NRANGE):
                            b0, b1 = spans[w][r]
                            nb = b1 - b0
                            if nb == 0:
                                continue
                            lc = b0 - c0
                            s_t = spool.tile([P, max_span, P],
                                             dt.bfloat16, tag="s_t")
                            nc.vector.tensor_tensor(
                                out=s_t[:, :nb, :],
                                in0=iota_sb[:, :, :].to_broadcast(
                                    [P, nb, P]),
                                in1=d_t[:, lc:lc + nb, wi:wi + 1]
                                .to_broadcast([P, nb, P]),
                                op=OP.is_equal)
                            for k in range(nb):
                                nc.tensor.matmul(
                                    agg[:], lhsT=s_t[:, k, :],
                                    rhs=g_t[:, lc + k, :],
                                    start=(done == 0),
                                    stop=(done == nb_w - 1))
                                done += 1
                        if l < 2:
                            t_t = epool.tile([P, dout], dt.bfloat16,
                                             tag="t_t")
                            nc.scalar.activation(t_t[:], agg[:, :dout],
                                                 AF.Copy,
                                                 scale=invd_sb[:, w:w + 1])
                            o_t = epool.tile([P, dout], dt.bfloat16,
                                             tag="o_t")
                            nc.vector.tensor_tensor(out=o_t[:], in0=t_t[:],
                                                    in1=r_res[:, w, :dout],
                                                    op=OP.add)
                            nc.vector.tensor_scalar_max(
                                h_sb[l % 2][:, w, :dout], o_t[:], 0.0)
                            if "dense" not in ablate and fuse_dense:
                                dense_w(l + 1, w,
                                        h_sb[l % 2][:, w, :dout])
                        else:
                            t_t = epool.tile([P, dout], dt.float32,
                                             tag="t_t")
                            nc.scalar.activation(t_t[:], agg[:, :dout],
                                                 AF.Copy,
                                                 scale=invd_sb[:, w:w + 1])
                            o_t = epool.tile([P, dout], dt.float32,
                                             tag="o_t")
                            nc.vector.tensor_tensor(out=o_t[:], in0=t_t[:],
                                                    in1=r_res[:, w, :dout],
                                                    op=OP.add)
                            nc.sync.dma_start(h_out[w * P:(w + 1) * P, :],
                                              o_t[:])


    nc.compile()
    return nc


def _preprocess(x, src, dst, ncores=NCORES, nloc=NLOC, nw=NW, nlocp=NLOCP,
                nsw=NSW):
    """Pack per-core edge/index arrays grouped by (super-window, src range).

    Edges are packed edge-granularly inside each (sw, range) run using
    SPMD-uniform per-(window, range) segment sizes (max over cores), so
    only the run total is padded to whole 128-slot blocks. A block may
    hold edges of several windows; the per-window one-hot selects them
    via per-window dstw planes (-1 for foreign slots).

    Returns (per_core input dicts, layout dict for _build_program).
    """
    bf16 = ml_dtypes.bfloat16
    nfullp = ncores * nlocp
    rsize = nfullp // NRANGE

    order = np.argsort(dst, kind="stable")
    src_s = src[order].astype(np.int64)
    dst_s = dst[order].astype(np.int64)
    bounds = np.searchsorted(dst_s, np.arange(ncores + 1) * nloc)

    cores = []
    cnts = np.zeros((ncores, nw, NRANGE), np.int64)
    for c in range(ncores):
        lo, hi = bounds[c], bounds[c + 1]
        s = src_s[lo:hi]
        lcl = dst_s[lo:hi] - c * nloc
        w = lcl // P
        sowner = s // nloc
        s_pad = sowner * nlocp + (s - sowner * nloc)
        rix = s_pad // rsize
        key = w * NRANGE + rix
        o2 = np.lexsort((s_pad, key))
        s_pad, lcl, w, rix, key = (s_pad[o2], lcl[o2], w[o2], rix[o2],
                                   key[o2])
        cnts[c] = np.bincount(key, minlength=nw * NRANGE)\
            .reshape(nw, NRANGE)
        cores.append((s_pad, lcl, w, rix, key))

    # SPMD-uniform segment sizes: max edge count over cores per (w, r)
    seg = cnts.max(axis=0).astype(np.int64)        # [nw, NRANGE]
    seg[:, 0] = np.maximum(seg[:, 0], 1)           # keep every window alive

    sw_groups = _sw_groups(nw, nsw)
    seg_start = np.zeros((nw, NRANGE), np.int64)   # slot offset within run
    run_col = np.zeros((len(sw_groups), NRANGE), np.int64)
    runs = []
    spans = [[None] * NRANGE for _ in range(nw)]   # (b0, b1) global cols
    sw_col_start = [0]
    gc = 0
    for si, grp in enumerate(sw_groups):
        sw_runs = []
        for r in range(NRANGE):
            tot = 0
            for w in grp:
                seg_start[w, r] = tot
                tot += seg[w, r]
            nb = (tot + P - 1) // P
            run_col[si, r] = gc
            for w in grp:
                s0, s1 = seg_start[w, r], seg_start[w, r] + seg[w, r]
                if s1 > s0:
                    spans[w][r] = (int(gc + s0 // P),
                                   int(gc + (s1 + P - 1) // P))
                else:
                    spans[w][r] = (int(gc), int(gc))
            sw_runs.append((r, int(gc), int(nb)))
            gc += nb
        runs.append(sw_runs)
        sw_col_start.append(int(gc))
    total_cols = int(gc)
    max_sw_cols = max(sw_col_start[i + 1] - sw_col_start[i]
                      for i in range(len(sw_groups)))
    max_span = max(b1 - b0 for row in spans for (b0, b1) in row)
    layout = {
        "seg": seg.tolist(),
        "runs": runs,
        "spans": spans,
        "sw_groups": sw_groups,
        "sw_col_start": sw_col_start,
        "total_cols": total_cols,
        "max_sw_cols": int(max_sw_cols),
        "max_span": int(max_span),
    }

    per_core = []
    for c in range(ncores):
        s_pad, lcl, w, rix, key = cores[c]
        cnt = cnts[c]
        starts = np.zeros(nw * NRANGE, np.int64)
        starts[1:] = np.cumsum(cnt.ravel())[:-1]
        j = np.arange(len(lcl)) - starts[key]
        sw = w // nsw
        slot_in_run = seg_start[w, rix] + j
        rcol = run_col[sw, rix]
        col = rcol + slot_in_run // P
        pp = slot_in_run % P
        # per-window dstw planes: plane wi holds the dst offset within
        # window (sw*nsw + wi) for that window's slots, -1 elsewhere
        dstw = np.full((P, total_cols, nsw), -1.0, np.float32)
        dstw[pp, col, w % nsw] = (lcl % P).astype(np.float32)
        i16col = rcol * 8 + slot_in_run // 16
        i16row = slot_in_run % 16
        idx16 = np.zeros((16, total_cols * 8), np.int16)
        idx16[i16row, i16col] = (s_pad - rix * rsize).astype(np.int16)
        idx16 = np.tile(idx16, (8, 1))
        deg = np.bincount(lcl, minlength=nlocp).astype(np.float32)
        invd = (1.0 / np.maximum(deg, 1.0)).reshape(nw, P).T.copy()
        x_pad = np.zeros((nlocp, x.shape[1]), np.float32)
        x_pad[:nloc] = x[c * nloc:(c + 1) * nloc]
        per_core.append({
            "x_local": x_pad,
            "idx16": idx16,
            "dstw": dstw.astype(bf16),
            "invd": invd.astype(np.float32),
        })
    return per_core, layout


def _run_pjrt(nc, in_maps, n_cores, bench_iters=0):
    """Execute the Bass program on the NeuronCores via PJRT/axon.

    Mirrors concourse.bass2jax.run_bass_via_pjrt, with an optional timing
    loop: inputs are pre-placed on device so repeated calls measure
    execute time (plus dispatch overhead) rather than host transfers.
    Returns (per_core_results, best_ns or None).
    """
    import time
    import jax
    import concourse.mybir as mybir
    from concourse.bass2jax import (_bass_exec_p, install_neuronx_cc_hook,
                                    partition_id_tensor)
    from jax.experimental.shard_map import shard_map
    from jax.sharding import Mesh, NamedSharding, PartitionSpec

    install_neuronx_cc_hook()

    partition_name = (nc.partition_id_tensor.name
                      if nc.partition_id_tensor else None)
    in_names, out_names, out_avals, zero_outs = [], [], [], []
    for alloc in nc.m.functions[0].allocations:
        if not isinstance(alloc, mybir.MemoryLocationSet):
            continue
        name = alloc.memorylocations[0].name
        if alloc.kind == "ExternalInput":
            if name != partition_name:
                in_names.append(name)
        elif alloc.kind == "ExternalOutput":
            shape = tuple(alloc.tensor_shape)
            dtype = mybir.dt.np(alloc.dtype)
            out_names.append(name)
            out_avals.append(jax.core.ShapedArray(shape, dtype))
            zero_outs.append(np.zeros(shape, dtype))
    n_params = len(in_names)
    n_outs = len(out_avals)
    in_names.extend(out_names)
    if partition_name is not None:
        in_names.append(partition_name)

    def _body(*args):
        operands = list(args)
        if partition_name is not None:
            operands.append(partition_id_tensor())
        return tuple(_bass_exec_p.bind(
            *operands,
            out_avals=tuple(out_avals),
            in_names=tuple(in_names),
            out_names=tuple(out_names),
            lowering_input_output_aliases=(),
            sim_require_finite=True,
            sim_require_nnan=True,
            nc=nc,
        ))

    devices = jax.devices()[:n_cores]
    assert len(devices) >= n_cores, devices
    mesh = Mesh(np.asarray(devices), ("core",))
    in_specs = (PartitionSpec("core"),) * (n_params + n_outs)
    out_specs = (PartitionSpec("core"),) * n_outs
    sharded = jax.jit(
        shard_map(_body, mesh=mesh, in_specs=in_specs, out_specs=out_specs,
                  check_rep=False),
        keep_unused=True)

    per_core = [[np.asarray(m[name]) for name in in_names[:n_params]]
                for m in in_maps]
    concat_in = [np.concatenate([per_core[c][i] for c in range(n_cores)],
                                axis=0) for i in range(n_params)]
    concat_zeros = [np.zeros((n_cores * z.shape[0], *z.shape[1:]), z.dtype)
                    for z in zero_outs]

    sharding = NamedSharding(mesh, PartitionSpec("core"))
    dev_in = [jax.device_put(a, sharding) for a in concat_in]
    dev_zeros = [jax.device_put(z, sharding) for z in concat_zeros]

    out_arrs = sharded(*dev_in, *dev_zeros)
    out_arrs = [np.asarray(o) for o in out_arrs]

    best_ns = None
    if bench_iters:
        # Device-time measurement through the axon tunnel: issue a chain
        # of async dispatches where each call's (unused) zero-output
        # operands come from the previous call's outputs. The device
        # serializes the NEFF executions while dispatch RPCs pipeline
        # (marginal dispatch cost ~0.2 ms/call at depth >=32), so
        # (t_deep - t_shallow) / (n_deep - n_shallow) isolates per-exec
        # device time and cancels the ~72 ms round-trip.
        def chain(n):
            outs = tuple(dev_zeros)
            t0 = time.perf_counter()
            for _ in range(n):
                outs = sharded(*dev_in, *outs)
            for r in outs:
                r.block_until_ready()
            return time.perf_counter() - t0

        n1, n2 = 12, 12 + max(24, 2 * bench_iters)
        chain(4)  # warm
        for _ in range(3):
            t1 = chain(n1)
            t2 = chain(n2)
            dt_ns = (t2 - t1) / (n2 - n1) * 1e9
            best_ns = dt_ns if best_ns is None else min(best_ns, dt_ns)

    results = [
        {name: out_arrs[i].reshape(n_cores, *out_avals[i].shape)[c]
         for i, name in enumerate(out_names)}
        for c in range(n_cores)
    ]
    return results, best_ns


def kernel(x, edge_index, Wl0, Wr0, b0, Wl1, Wr1, b1, Wl2, Wr2, b2):
    global LAST_EXEC_TIME_NS, LAST_RESULTS

    bf16 = ml_dtypes.bfloat16
    x = np.ascontiguousarray(np.asarray(x, np.float32))
    ei = np.asarray(edge_index)
    src = ei[0].astype(np.int64)
    dst = ei[1].astype(np.int64)

    per_core, layout = _preprocess(x, src, dst)

    Ws = [(np.asarray(Wl0, np.float32), np.asarray(Wr0, np.float32),
           np.asarray(b0, np.float32)),
          (np.asarray(Wl1, np.float32), np.asarray(Wr1, np.float32),
           np.asarray(b1, np.float32)),
          (np.asarray(Wl2, np.float32), np.asarray(Wr2, np.float32),
           np.asarray(b2, np.float32))]
    shared = {}
    for l, (Wl, Wr, b) in enumerate(Ws):
        shared[f"wcat{l}"] = np.ascontiguousarray(
            np.concatenate([Wl, Wr], axis=1).astype(bf16))
        shared[f"bbc{l}"] = np.ascontiguousarray(
            np.tile(b[None, :], (P, 1)).astype(np.float32))
    shared["iota"] = np.tile(np.arange(P, dtype=np.float32)[None, None, :],
                             (P, 1, 1)).astype(bf16)

    in_maps = [{**pc, **shared} for pc in per_core]
    for m in in_maps:
        m["x_local"] = m["x_local"].astype(bf16)

    nc = _build_program(layout)
    bench_iters = int(os.environ.get("GSAGE_BENCH_ITERS", "8"))
    results, best_ns = _run_pjrt(nc, in_maps, NCORES,
                                 bench_iters=bench_iters)
    LAST_EXEC_TIME_NS = best_ns
    LAST_RESULTS = results

    out = np.empty((N_NODES, OUT_CH), np.float32)
    for c in range(NCORES):
        out[c * NLOC:(c + 1) * NLOC] = results[c]["h_out"][:NLOC]
    return out

